# revision 12
# baseline (speedup 1.0000x reference)
"""Trainium2 Bass kernel for the attention layer:

    f = wf@x+bf; g = wg@x+bg; h = wh@x+bh            (1x1 convs, Ci=32)
    attn = softmax(f^T g, axis=-1)                   (per batch, N=4096)
    out = (wv @ (h @ attn^T) + bv) * gamma + x

Sharding: 8 cores = 4 batches x 2 query-halves (2048 queries each).
Each core receives the full (256, 4096) batch slice with its query half
permuted to the front, so the SPMD program uses fixed offsets.

Per-core dataflow (all-bf16 matmuls, PSUM fp32 accumulate):
  - x arrives bf16 only (2MB); the residual is added from the bf16 copy
    via an IDENTITY MATMUL accumulated into the projection PSUM bank,
    so the output copy is a pure PSUM->SBUF copy (balanceable between
    ACT and DVE) instead of a DVE-only tensor_tensor add.
  - exp is the elementwise bottleneck (32*2048 partition-cycles/core)
    and GPSIMD/DMA cannot touch PSUM, so every PSUM consumer is either
    ACT or DVE: exp groups alternate ACT (true exp, PSUM->bf16) and DVE
    (Schraudolph fast-exp: bf16 bits = int16(l*128/ln2 + 16250) in one
    tensor_scalar, ~3.5% rel err); PSUM->SBUF copies alternate engines;
    softmax divide uses reciprocal_approx_fast.
  - PSUM budget (8 banks): logits pool 3 bufs x 2 banks so the logits
    matmul for group g WAR-waits exp(g-3), letting both exp engines run
    fully parallel; 1 bank for projections/warmup/fillers; 1 bank for
    the x0 accumulator.
  - the whole f/g/hT prologue runs before the chunk loop, cycling
    through the logits pool buffers; hT lands 16 key-chunks per PSUM
    bank ([128,16,32] fp32 = 2KB) so one big copy replaces 16 small.
  - per 512-query chunk: 32 row-packed logits matmuls (strip kc%4,
    concurrent PE row bands) -> exp groups of 2 k-chunks -> 32
    x0-accumulation matmuls (hT stationary, ones column in row 0
    accumulating the softmax denominator).
  - projection+residual+DMA for chunk i are DEFERRED into chunk i+1's
    group loop (two halves at different groups so PSUM WARs stay off
    the PE's critical path); dummy 512-row matmuls pad PE idle slots so
    the HAM clock gate (K/N pulse gating, 1.2 vs 2.4 GHz) stays open.
"""

import os
import numpy as np
import ml_dtypes

import concourse.bass as bass
import concourse.mybir as mybir
import concourse.tile as tile
from concourse import bacc
from concourse.bass import ts
from concourse.bass_utils import run_bass_kernel_spmd

F32 = mybir.dt.float32
F32R = mybir.dt.float32r
BF16 = mybir.dt.bfloat16
I16 = mybir.dt.int16
EXP = mybir.ActivationFunctionType.Exp
IDENT = mybir.ActivationFunctionType.Identity
MUL = mybir.AluOpType.mult
ADD = mybir.AluOpType.add

B, C, W, H = 4, 256, 64, 64
N = W * H            # 4096 keys/queries per batch
CI = 32              # inner channels
NCORES = 8
NQ = N // 2          # queries per core
QC = 512             # query chunk = one fp32 PSUM bank
NQC = NQ // QC       # 4 query chunks per core
KC = 128             # key chunk = partition dim
NKC = N // KC        # 32 key chunks
GRP = 2              # key chunks per exp group (PSUM banks per tile)
NGRP = NKC // GRP    # 16 groups per chunk
PIPE = 2             # software-pipeline depth (groups) for x0 stage
NWARM = 4            # dummy bf16 matmuls to warm the PE clock gate
FILL_EVERY = 8       # PE filler matmul every this many groups
NSLICE = 4           # x DMA slices

NPAIR = NQC // 2     # query chunks are processed in fused pairs
LAG = 4              # key-chunks between exp and x0 consumption

# Schraudolph fast-exp constants (bf16 bits = int16(l*EXP_A + EXP_B))
EXP_A = 128.0 / float(np.log(2.0))
EXP_B = 16250.0
# key-chunks handled by DVE fast-exp (rest go to ACT true exp): 14 of 32
DVE_KCS = frozenset(kc for kc in range(NKC) if kc % 2 == 1 and kc < 28)

# Trace knob for test harnesses: set kernel.TRACE = True to profile.
TRACE = False
LAST_EXEC_NS = None

_cached_nc = None


def _mm(nc, out, lhsT, rhs, start, stop, tile_position=None):
    nc.tensor.matmul(out, lhsT=lhsT, rhs=rhs, start=start, stop=stop,
                     tile_position=tile_position)


def _build():
    nc = bacc.Bacc(
        "TRN2", target_bir_lowering=False, debug=False, num_devices=NCORES
    )
    xbf_d = nc.dram_tensor("xbf", (C, N), BF16, kind="ExternalInput").ap()
    wfT_d = nc.dram_tensor("wfT", (C, 128), BF16, kind="ExternalInput").ap()
    wgT_d = nc.dram_tensor("wgT", (C, 128), BF16, kind="ExternalInput").ap()
    whT_d = nc.dram_tensor("whT", (C, CI), BF16, kind="ExternalInput").ap()
    wvT_d = nc.dram_tensor("wvT", (CI + 1, C), BF16, kind="ExternalInput").ap()
    wid_d = nc.dram_tensor("wid", (128, 128), BF16, kind="ExternalInput").ap()
    bf_d = nc.dram_tensor("bf", (128, 1), F32, kind="ExternalInput").ap()
    bg_d = nc.dram_tensor("bg", (128, 1), F32, kind="ExternalInput").ap()
    out_d = nc.dram_tensor("out", (C, NQ), F32, kind="ExternalOutput").ap()

    outr = out_d.rearrange("(oc p) n -> p oc n", p=128)
    xbfr = xbf_d.rearrange("(cc p) n -> p cc n", p=128)

    with tile.TileContext(nc) as tc:
        with (
            tc.tile_pool(name="consts", bufs=1) as consts,
            tc.tile_pool(name="data", bufs=1) as data,
            tc.tile_pool(name="eTp", bufs=6) as eTp,
            tc.tile_pool(name="smallp", bufs=2) as smallp,
            tc.tile_pool(name="outp", bufs=3) as outp,
            tc.tile_pool(name="pl", bufs=2, space="PSUM") as pl,
            tc.tile_pool(name="pxB", bufs=1, space="PSUM") as pxB,
            tc.tile_pool(name="pp", bufs=1, space="PSUM") as pp,
            tc.tile_pool(name="px0", bufs=1, space="PSUM") as px0,
        ):
            # ---- PE + ACT warm-up (overlaps the input DMAs) ----
            scratch = consts.tile([128, QC], BF16)
            nc.vector.memset(scratch, 0.0)

            def fill(n=1):
                # HAM filler: keeps the PE streaming through slots where
                # it would otherwise idle (and re-throttle to 1.2 GHz).
                wps = pp.tile([128, QC], F32, tag="pp", name="wps")
                for _ in range(n):
                    nc.tensor.matmul(
                        wps, lhsT=scratch[:, 0:128], rhs=scratch,
                        start=True, stop=True, skip_group_check=True,
                    )

            fill(NWARM)
            scratch2 = consts.tile([1, 8], F32)
            nc.scalar.activation(out=scratch2, in_=scratch[0:1, 0:8], func=EXP)

            # ---- constants ----
            wfT_sb = consts.tile([128, 2, 128], BF16)
            nc.sync.dma_start(
                out=wfT_sb, in_=wfT_d.rearrange("(cc p) o -> p cc o", p=128)
            )
            wgT_sb = consts.tile([128, 2, 128], BF16)
            nc.sync.dma_start(
                out=wgT_sb, in_=wgT_d.rearrange("(cc p) o -> p cc o", p=128)
            )
            whT_sb = consts.tile([128, 2, CI], BF16)
            nc.sync.dma_start(
                out=whT_sb, in_=whT_d.rearrange("(cc p) o -> p cc o", p=128)
            )
            wvT_sb = consts.tile([CI + 1, 2, 128], BF16)
            nc.sync.dma_start(
                out=wvT_sb, in_=wvT_d.rearrange("p (oc m) -> p oc m", oc=2)
            )
            wid_sb = consts.tile([128, 128], BF16)
            nc.sync.dma_start(out=wid_sb, in_=wid_d)
            bf_sb = consts.tile([128, 1], F32)
            nc.sync.dma_start(out=bf_sb, in_=bf_d)
            bg_sb = consts.tile([128, 1], F32)
            nc.sync.dma_start(out=bg_sb, in_=bg_d)
            ones_sb = consts.tile([128, 1], BF16)
            nc.vector.memset(ones_sb, 1.0)

            # ---- x (bf16, 4 column slices so compute starts early) ----
            xbf_sb = data.tile([128, 2, N], BF16)
            for s in range(NSLICE):
                nc.sync.dma_start(
                    out=xbf_sb[:, :, ts(s, N // NSLICE)],
                    in_=xbfr[:, :, ts(s, N // NSLICE)],
                )

            # ---- f, g (replicated on 4 strips), hT ----
            f_sb = data.tile([128, NQ], BF16)
            g_sb = data.tile([128, N], BF16)
            hT_sb = data.tile([128, NKC, CI + 1], BF16)
            nc.vector.tensor_copy(
                hT_sb[:, :, 0:1], ones_sb.to_broadcast([128, NKC, 1])
            )

            # prologue blocks cycle through the logits pool's 3 buffers;
            # PSUM->SBUF copy work alternates between ACT and DVE.
            def emit_fg(dst, w_sb, b_sb, j, on_act):
                ps = pl.tile([128, GRP, QC], F32, tag="lg", name=f"fg{j}")
                for cc in range(2):
                    _mm(nc, ps[:, 0, :], w_sb[:, cc, :],
                        xbf_sb[:, cc, ts(j, QC)], cc == 0, cc == 1)
                if on_act:
                    nc.scalar.activation(
                        out=dst[:, ts(j, QC)], in_=ps[:, 0, :], func=IDENT,
                        bias=b_sb,
                    )
                else:
                    nc.vector.tensor_scalar_add(
                        dst[:, ts(j, QC)], ps[:, 0, :], b_sb
                    )

            # hT production: one PSUM bank holds 16 key-chunks
            # ([128,16,32] fp32 = 2KB/partition), one big copy each.
            def emit_hT(half, on_act):
                hps = pl.tile([128, 16, CI], F32, tag="lg", name=f"ph{half}")
                for sl in range(16):
                    kc = 16 * half + sl
                    for cc in range(2):
                        _mm(nc, hps[:, sl, :], xbf_sb[:, cc, ts(kc, KC)],
                            whT_sb[:, cc, :], cc == 0, cc == 1)
                dst = hT_sb[:, 16 * half : 16 * (half + 1), 1 : CI + 1]
                if on_act:
                    nc.scalar.copy(dst, hps)
                else:
                    nc.vector.tensor_copy(dst, hps)

            # ---- prologue: f (own queries), g + hT (all keys) ----
            for j in range(4):
                emit_fg(f_sb, wfT_sb, bf_sb, j, on_act=(j % 2 == 0))
                emit_fg(g_sb, wgT_sb, bg_sb, j, on_act=(j % 2 == 1))
                fill(1)
            emit_hT(0, on_act=True)
            for j in range(4, 8):
                emit_fg(g_sb, wgT_sb, bg_sb, j, on_act=(j % 2 == 1))
                fill(1)
            emit_hT(1, on_act=False)

            # deferred projection + residual + output for chunk qi
            x0a_by_chunk = {}

            def emit_out(qi, oc, tail=False):
                x0a = x0a_by_chunk[qi]
                if tail and oc == 1:
                    # final chunk: second projection borrows a logits
                    # bank so both output copies run concurrently.
                    big = pl.tile([128, GRP, QC], F32, tag="lg", name="pst")
                    vps = big[:, 0, :]
                else:
                    vps = pp.tile([128, QC], F32, tag="pp",
                                  name=f"psv{qi}{oc}")
                _mm(nc, vps, wvT_sb[:, oc, :], x0a, True, False)
                # residual: + I @ x  (identity matmul accumulate)
                _mm(nc, vps, wid_sb, xbf_sb[:, oc, ts(qi, QC)], False, True)
                ot = outp.tile([128, QC], F32)
                if oc == 0:
                    nc.scalar.copy(ot, vps)
                else:
                    nc.vector.tensor_copy(ot, vps)
                nc.sync.dma_start(out=outr[:, oc, ts(qi, QC)], in_=ot)

            # ---- main loop over fused query-chunk pairs ----
            # One 1024-row logits matmul per key-chunk covers both
            # chunks of the pair; one exp instruction; the x0 pair
            # accumulates in a single PSUM bank ([97,512]: rows 0-32 =
            # chunk A, rows 64-96 = chunk B via tile_position col 64).
            def emit_x0(x0t, kc, eT):
                x0tA, x0tB = x0t
                _mm(nc, x0tA, hT_sb[:, kc, :], eT[:, 0, :],
                    kc == 0, kc == NKC - 1)
                _mm(nc, x0tB, hT_sb[:, kc, :], eT[:, 1, :],
                    kc == 0, kc == NKC - 1)

            for p in range(NPAIR):
                qA, qB = 2 * p, 2 * p + 1
                x0t = (px0.tile([CI + 1, QC], F32, name='x0tA'),
                       pxB.tile([CI + 1, QC], F32, name='x0tB'))
                x0q = []
                for kc in range(NKC):
                    ps = pl.tile([128, GRP, QC], F32, tag="lg")
                    eT = eTp.tile([128, GRP, QC], BF16)
                    # row-packed: strip kc%4 holds its own copy of g/f,
                    # so adjacent key-chunks execute concurrently in
                    # different PE row bands; the pair's second matmul
                    # reuses the loaded stationary (no LS, no wait).
                    s = kc % 4
                    sl = slice(32 * s, 32 * (s + 1))
                    for j, qi in enumerate((qA, qB)):
                        nc.tensor.matmul(
                            ps[:, j, :],
                            lhsT=g_sb[sl, ts(kc, KC)],
                            rhs=f_sb[sl, ts(qi, QC)],
                            start=True, stop=True,
                            tile_position=(32 * s, 0),
                        )
                    if kc in DVE_KCS:
                        # Schraudolph fast-exp on DVE: bf16 bits of
                        # exp(l) ~= int16(l*EXP_A + EXP_B)
                        nc.vector.tensor_scalar(
                            out=eT.bitcast(I16), in0=ps,
                            scalar1=EXP_A, scalar2=EXP_B,
                            op0=MUL, op1=ADD,
                        )
                    else:
                        nc.scalar.activation(out=eT, in_=ps, func=EXP)
                    # software-pipeline the x0 stage: its wait on this
                    # key-chunk's exp then overlaps later logits in the
                    # in-order PE stream.
                    x0q.append((kc, eT))
                    if len(x0q) > LAG:
                        pkc, peT = x0q.pop(0)
                        emit_x0(x0t, pkc, peT)
                    if kc % FILL_EVERY == FILL_EVERY - 1:
                        fill(1)
                    if p > 0:
                        if kc == 6:
                            emit_out(qA - 2, 0)
                        elif kc == 10:
                            emit_out(qA - 2, 1)
                        elif kc == 14:
                            emit_out(qB - 2, 0)
                        elif kc == 18:
                            emit_out(qB - 2, 1)
                for pkc, peT in x0q:
                    emit_x0(x0t, pkc, peT)
                # softmax divide: row 0/64 hold the denominators
                for half, qi in ((0, qA), (1, qB)):
                    xh = x0t[half]
                    rcp = smallp.tile([1, QC], F32, tag="rcp")
                    nc.vector.reciprocal_approx_fast(
                        out=rcp, in_=xh[0:1, :]
                    )
                    rcp_b = smallp.tile([CI + 1, QC], F32, tag="rcpb")
                    nc.gpsimd.partition_broadcast(rcp_b, rcp)
                    x0a = smallp.tile([CI + 1, QC], BF16, tag="x0a")
                    nc.vector.tensor_mul(x0a, xh, rcp_b)
                    x0a_by_chunk[qi] = x0a
            emit_out(NQC - 2, 0)
            emit_out(NQC - 2, 1)
            emit_out(NQC - 1, 0, tail=True)
            emit_out(NQC - 1, 1, tail=True)

    nc.compile()
    return nc


def kernel(x, wf, bf, wg, bg, wh, bh, wv, bv, gamma):
    global _cached_nc, LAST_EXEC_NS
    if _cached_nc is None:
        _cached_nc = _build()
    nc = _cached_nc

    x = np.asarray(x, dtype=np.float32)
    wf = np.asarray(wf, dtype=np.float32)
    bf = np.asarray(bf, dtype=np.float32)
    wg = np.asarray(wg, dtype=np.float32)
    bg = np.asarray(bg, dtype=np.float32)
    wh = np.asarray(wh, dtype=np.float32)
    bh = np.asarray(bh, dtype=np.float32)
    wv = np.asarray(wv, dtype=np.float32)
    bv = np.asarray(bv, dtype=np.float32)
    g0 = float(np.asarray(gamma, dtype=np.float32).reshape(-1)[0])

    bf16 = ml_dtypes.bfloat16
    xf = np.ascontiguousarray(x.reshape(B, C, N))
    # f/g weights replicated 4x along M so f/g land replicated on the
    # four 32-partition strips (enables row-packed logits matmuls).
    wfT = np.ascontiguousarray(np.tile(wf.T, (1, 4))).astype(bf16)
    wgT = np.ascontiguousarray(np.tile(wg.T, (1, 4))).astype(bf16)
    whT = np.ascontiguousarray(wh.T).astype(bf16)
    wvT = np.empty((CI + 1, C), np.float32)              # aug: bias row 0
    wvT[0, :] = g0 * (bv + wv @ bh)
    wvT[1:, :] = g0 * wv.T
    wvT = wvT.astype(bf16)
    wid = np.eye(128, dtype=np.float32).astype(bf16)
    bf4 = np.ascontiguousarray(np.tile(bf, 4).reshape(128, 1))
    bg4 = np.ascontiguousarray(np.tile(bg, 4).reshape(128, 1))

    in_maps = []
    for core in range(NCORES):
        b, half = divmod(core, 2)
        xb = xf[b]
        if half:
            xb = np.concatenate([xb[:, NQ:], xb[:, :NQ]], axis=1)
        in_maps.append(
            {"xbf": np.ascontiguousarray(xb.astype(bf16)), "wfT": wfT,
             "wgT": wgT, "whT": whT, "wvT": wvT, "wid": wid,
             "bf": bf4, "bg": bg4}
        )

    res = run_bass_kernel_spmd(
        nc, in_maps, list(range(NCORES)),
        trace=TRACE or bool(os.environ.get("BASS_KERNEL_TRACE")),
    )
    LAST_EXEC_NS = res.exec_time_ns

    out = np.empty((B, C, N), np.float32)
    for core in range(NCORES):
        b, half = divmod(core, 2)
        out[b][:, half * NQ : (half + 1) * NQ] = res.results[core]["out"]
    return out.reshape(B, C, W, H)


# revision 13
# speedup vs baseline: 1.2359x; 1.2359x over previous
"""Trainium2 Bass kernel for the attention layer:

    f = wf@x+bf; g = wg@x+bg; h = wh@x+bh            (1x1 convs, Ci=32)
    attn = softmax(f^T g, axis=-1)                   (per batch, N=4096)
    out = (wv @ (h @ attn^T) + bv) * gamma + x

Sharding: 8 cores = 4 batches x 2 query-halves (2048 queries each).
Each core receives the full (256, 4096) batch slice with its query half
permuted to the front, so the SPMD program uses fixed offsets.

Per-core dataflow (all-bf16 matmuls, PSUM fp32 accumulate):
  - x arrives bf16 only (2MB); the residual is added from the bf16 copy
    via an IDENTITY MATMUL accumulated into the projection PSUM bank,
    so the output copy is a pure PSUM->SBUF copy (balanceable between
    ACT and DVE) instead of a DVE-only tensor_tensor add.
  - exp is the elementwise bottleneck (32*2048 partition-cycles/core)
    and GPSIMD/DMA cannot touch PSUM, so every PSUM consumer is either
    ACT or DVE: exp groups alternate ACT (true exp, PSUM->bf16) and DVE
    (Schraudolph fast-exp: bf16 bits = int16(l*128/ln2 + 16250) in one
    tensor_scalar, ~3.5% rel err); PSUM->SBUF copies alternate engines;
    softmax divide uses reciprocal_approx_fast.
  - PSUM budget (8 banks): logits pool 3 bufs x 2 banks so the logits
    matmul for group g WAR-waits exp(g-3), letting both exp engines run
    fully parallel; 1 bank for projections/warmup/fillers; 1 bank for
    the x0 accumulator.
  - tiny 1x1 "absorber" matmuls carry the cross-engine semaphore waits
    so the real 512-row matmuls issue back-to-back with their weight
    loads prefetched (an exposed wait blocks LS double-buffering and
    costs ~100ns per matmul).
  - the whole f/g/hT prologue runs before the chunk loop, cycling
    through the logits pool buffers; hT lands 16 key-chunks per PSUM
    bank ([128,16,32] fp32 = 2KB) so one big copy replaces 16 small.
  - per 512-query chunk: 32 row-packed logits matmuls (strip kc%4,
    concurrent PE row bands) -> exp groups of 2 k-chunks -> 32
    x0-accumulation matmuls (hT stationary, ones column in row 0
    accumulating the softmax denominator).
  - projection+residual+DMA for chunk i are DEFERRED into chunk i+1's
    group loop; occasional dummy 512-row matmuls pad PE idle slots so
    the HAM clock gate (K/N pulse gating, 1.2 vs 2.4 GHz) stays open.
"""

import os
import numpy as np
import ml_dtypes

import concourse.bass as bass
import concourse.mybir as mybir
import concourse.tile as tile
from concourse import bacc
from concourse.bass import ts
from concourse.bass_utils import run_bass_kernel_spmd

F32 = mybir.dt.float32
F32R = mybir.dt.float32r
BF16 = mybir.dt.bfloat16
I16 = mybir.dt.int16
EXP = mybir.ActivationFunctionType.Exp
IDENT = mybir.ActivationFunctionType.Identity
MUL = mybir.AluOpType.mult
ADD = mybir.AluOpType.add

B, C, W, H = 4, 256, 64, 64
N = W * H            # 4096 keys/queries per batch
CI = 32              # inner channels
NCORES = 8
NQ = N // 2          # queries per core
QC = 512             # query chunk = one fp32 PSUM bank
NQC = NQ // QC       # 4 query chunks per core
KC = 128             # key chunk = partition dim
NKC = N // KC        # 32 key chunks
GRP = 2              # key chunks per exp group (PSUM banks per tile)
NGRP = NKC // GRP    # 16 groups per chunk
PIPE = 2             # software-pipeline depth (groups) for x0 stage
NWARM = 4            # dummy bf16 matmuls to warm the PE clock gate
FILL_EVERY = 8       # PE filler matmul every this many groups
NSLICE = 4           # x DMA slices

# Schraudolph fast-exp constants (bf16 bits = int16(l*EXP_A + EXP_B))
EXP_A = 128.0 / float(np.log(2.0))
EXP_B = 16250.0
# groups handled by DVE fast-exp (rest go to ACT true exp): 7 of 16
DVE_GROUPS = frozenset({1, 3, 5, 7, 9, 11, 13})

# Trace knob for test harnesses: set kernel.TRACE = True to profile.
TRACE = False
LAST_EXEC_NS = None

_cached_nc = None


def _mm(nc, out, lhsT, rhs, start, stop, tile_position=None):
    nc.tensor.matmul(out, lhsT=lhsT, rhs=rhs, start=start, stop=stop,
                     tile_position=tile_position)


def _build():
    nc = bacc.Bacc(
        "TRN2", target_bir_lowering=False, debug=False, num_devices=NCORES
    )
    xbf_d = nc.dram_tensor("xbf", (C, N), BF16, kind="ExternalInput").ap()
    wfT_d = nc.dram_tensor("wfT", (C, 128), BF16, kind="ExternalInput").ap()
    wgT_d = nc.dram_tensor("wgT", (C, 128), BF16, kind="ExternalInput").ap()
    whT_d = nc.dram_tensor("whT", (C, CI), BF16, kind="ExternalInput").ap()
    wvT_d = nc.dram_tensor("wvT", (CI + 1, C), BF16, kind="ExternalInput").ap()
    wid_d = nc.dram_tensor("wid", (128, 128), BF16, kind="ExternalInput").ap()
    bf_d = nc.dram_tensor("bf", (128, 1), F32, kind="ExternalInput").ap()
    bg_d = nc.dram_tensor("bg", (128, 1), F32, kind="ExternalInput").ap()
    out_d = nc.dram_tensor("out", (C, NQ), F32, kind="ExternalOutput").ap()

    outr = out_d.rearrange("(oc p) n -> p oc n", p=128)
    xbfr = xbf_d.rearrange("(cc p) n -> p cc n", p=128)

    with tile.TileContext(nc) as tc:
        with (
            tc.tile_pool(name="consts", bufs=1) as consts,
            tc.tile_pool(name="data", bufs=1) as data,
            tc.tile_pool(name="eTp", bufs=6) as eTp,
            tc.tile_pool(name="smallp", bufs=2) as smallp,
            tc.tile_pool(name="outp", bufs=3) as outp,
            tc.tile_pool(name="pl", bufs=3, space="PSUM") as pl,
            tc.tile_pool(name="pp", bufs=1, space="PSUM") as pp,
            tc.tile_pool(name="px0", bufs=1, space="PSUM") as px0,
        ):
            # ---- PE + ACT warm-up (overlaps the input DMAs) ----
            scratch = consts.tile([128, QC], BF16)
            nc.vector.memset(scratch, 0.0)

            def fill(n=1):
                # HAM filler: keeps the PE streaming through slots where
                # it would otherwise idle (and re-throttle to 1.2 GHz).
                wps = pp.tile([128, QC], F32, tag="pp", name="wps")
                for _ in range(n):
                    nc.tensor.matmul(
                        wps, lhsT=scratch[:, 0:128], rhs=scratch,
                        start=True, stop=True, skip_group_check=True,
                    )

            fill(NWARM)
            scratch2 = consts.tile([1, 8], F32)
            nc.scalar.activation(out=scratch2, in_=scratch[0:1, 0:8], func=EXP)

            # ---- constants ----
            wfT_sb = consts.tile([128, 2, 128], BF16)
            nc.sync.dma_start(
                out=wfT_sb, in_=wfT_d.rearrange("(cc p) o -> p cc o", p=128)
            )
            wgT_sb = consts.tile([128, 2, 128], BF16)
            nc.sync.dma_start(
                out=wgT_sb, in_=wgT_d.rearrange("(cc p) o -> p cc o", p=128)
            )
            whT_sb = consts.tile([128, 2, CI], BF16)
            nc.sync.dma_start(
                out=whT_sb, in_=whT_d.rearrange("(cc p) o -> p cc o", p=128)
            )
            wvT_sb = consts.tile([CI + 1, 2, 128], BF16)
            nc.sync.dma_start(
                out=wvT_sb, in_=wvT_d.rearrange("p (oc m) -> p oc m", oc=2)
            )
            wid_sb = consts.tile([128, 128], BF16)
            nc.sync.dma_start(out=wid_sb, in_=wid_d)
            bf_sb = consts.tile([128, 1], F32)
            nc.sync.dma_start(out=bf_sb, in_=bf_d)
            bg_sb = consts.tile([128, 1], F32)
            nc.sync.dma_start(out=bg_sb, in_=bg_d)
            ones_sb = consts.tile([128, 1], BF16)
            nc.vector.memset(ones_sb, 1.0)

            # ---- x (bf16, 4 column slices so compute starts early) ----
            xbf_sb = data.tile([128, 2, N], BF16)
            for s in range(NSLICE):
                nc.sync.dma_start(
                    out=xbf_sb[:, :, ts(s, N // NSLICE)],
                    in_=xbfr[:, :, ts(s, N // NSLICE)],
                )

            # ---- f, g (replicated on 4 strips), hT ----
            f_sb = data.tile([128, NQ], BF16)
            g_sb = data.tile([128, N], BF16)
            hT_sb = data.tile([128, NKC, CI + 1], BF16)
            nc.vector.tensor_copy(
                hT_sb[:, :, 0:1], ones_sb.to_broadcast([128, NKC, 1])
            )

            # prologue blocks cycle through the logits pool's 3 buffers;
            # PSUM->SBUF copy work alternates between ACT and DVE.
            def emit_fg(dst, w_sb, b_sb, j, on_act):
                ps = pl.tile([128, GRP, QC], F32, tag="lg", name=f"fg{j}")
                for cc in range(2):
                    _mm(nc, ps[:, 0, :], w_sb[:, cc, :],
                        xbf_sb[:, cc, ts(j, QC)], cc == 0, cc == 1)
                if on_act:
                    nc.scalar.activation(
                        out=dst[:, ts(j, QC)], in_=ps[:, 0, :], func=IDENT,
                        bias=b_sb,
                    )
                else:
                    nc.vector.tensor_scalar_add(
                        dst[:, ts(j, QC)], ps[:, 0, :], b_sb
                    )

            # hT production: one PSUM bank holds 16 key-chunks
            # ([128,16,32] fp32 = 2KB/partition), one big copy each.
            def emit_hT(half, on_act):
                hps = pl.tile([128, 16, CI], F32, tag="lg", name=f"ph{half}")
                for sl in range(16):
                    kc = 16 * half + sl
                    for cc in range(2):
                        _mm(nc, hps[:, sl, :], xbf_sb[:, cc, ts(kc, KC)],
                            whT_sb[:, cc, :], cc == 0, cc == 1)
                dst = hT_sb[:, 16 * half : 16 * (half + 1), 1 : CI + 1]
                if on_act:
                    nc.scalar.copy(dst, hps)
                else:
                    nc.vector.tensor_copy(dst, hps)

            # ---- prologue: f (own queries), g + hT (all keys) ----
            for j in range(4):
                emit_fg(f_sb, wfT_sb, bf_sb, j, on_act=(j % 2 == 0))
                emit_fg(g_sb, wgT_sb, bg_sb, j, on_act=(j % 2 == 1))
                fill(1)
            emit_hT(0, on_act=True)
            for j in range(4, 8):
                emit_fg(g_sb, wgT_sb, bg_sb, j, on_act=(j % 2 == 1))
                fill(1)
            emit_hT(1, on_act=False)

            # deferred projection + residual + output for chunk qi
            x0a_by_chunk = {}

            def emit_out(qi, oc, tail=False):
                x0a = x0a_by_chunk[qi]
                if tail and oc == 1:
                    # final chunk: second projection borrows a logits
                    # bank so both output copies run concurrently.
                    big = pl.tile([128, GRP, QC], F32, tag="lg", name="pst")
                    vps = big[:, 0, :]
                else:
                    vps = pp.tile([128, QC], F32, tag="pp",
                                  name=f"psv{qi}{oc}")
                _mm(nc, vps, wvT_sb[:, oc, :], x0a, True, False)
                # residual: + I @ x  (identity matmul accumulate)
                _mm(nc, vps, wid_sb, xbf_sb[:, oc, ts(qi, QC)], False, True)
                ot = outp.tile([128, QC], F32)
                if oc == 0:
                    nc.scalar.copy(ot, vps)
                else:
                    nc.vector.tensor_copy(ot, vps)
                nc.sync.dma_start(out=outr[:, oc, ts(qi, QC)], in_=ot)

            # ---- main loop over query chunks ----
            for qi in range(NQC):
                # row 0: softmax denominator (ones column in hT);
                # rows 1-32: x0 channels.
                x0 = px0.tile([CI + 1, QC], F32)
                x0q = []
                for gi, g0 in enumerate(range(0, NKC, GRP)):
                    ps = pl.tile([128, GRP, QC], F32, tag="lg")
                    eT = eTp.tile([128, GRP, QC], BF16)
                    # absorber: a 1x2 matmul takes the WAR wait on this
                    # logits buffer so the real matmuls issue wait-free
                    # with their weight loads prefetched.
                    nc.tensor.matmul(
                        ps[0:1, 0, 0:2], lhsT=scratch[0:1, 0:1],
                        rhs=scratch[0:1, 0:2], start=True, stop=True,
                        skip_group_check=True,
                    )
                    for j in range(GRP):
                        kc = g0 + j
                        # row-packed: strip kc%4 holds its own copy of
                        # g/f, so adjacent matmuls execute concurrently
                        # in different PE row bands.
                        s = kc % 4
                        sl = slice(32 * s, 32 * (s + 1))
                        nc.tensor.matmul(
                            ps[:, j, :],
                            lhsT=g_sb[sl, ts(kc, KC)],
                            rhs=f_sb[sl, ts(qi, QC)],
                            start=True, stop=True,
                            tile_position=(32 * s, 0),
                        )
                    if gi in DVE_GROUPS:
                        # Schraudolph fast-exp on DVE: bf16 bits of
                        # exp(l) ~= int16(l*EXP_A + EXP_B)
                        nc.vector.tensor_scalar(
                            out=eT.bitcast(I16), in0=ps,
                            scalar1=EXP_A, scalar2=EXP_B,
                            op0=MUL, op1=ADD,
                        )
                    else:
                        nc.scalar.activation(out=eT, in_=ps, func=EXP)
                    # software-pipeline the x0 stage: its wait on this
                    # group's exp then overlaps later groups' logits in
                    # the in-order PE stream.
                    x0q.append((g0, eT))
                    if len(x0q) > PIPE:
                        pg0, peT = x0q.pop(0)
                        # absorber for the exp->x0 wait
                        wps = pp.tile([128, QC], F32, tag="pp", name="wpsa")
                        nc.tensor.matmul(
                            wps[0:1, 0:2], lhsT=peT[0:1, 0, 0:1],
                            rhs=peT[0:1, 0, 0:2], start=True, stop=True,
                            skip_group_check=True,
                        )
                        for j in range(GRP):
                            kc = pg0 + j
                            _mm(nc, x0, hT_sb[:, kc, :], peT[:, j, :],
                                kc == 0, kc == NKC - 1)
                    if gi % FILL_EVERY == FILL_EVERY - 1:
                        fill(1)
                    if qi > 0:
                        if gi == 3:
                            emit_out(qi - 1, 0)
                        elif gi == 6:
                            emit_out(qi - 1, 1)
                            del x0a_by_chunk[qi - 1]
                for pg0, peT in x0q:
                    for j in range(GRP):
                        kc = pg0 + j
                        _mm(nc, x0, hT_sb[:, kc, :], peT[:, j, :],
                            kc == 0, kc == NKC - 1)
                # softmax divide: row 0 of x0 is the denominator
                rcp = smallp.tile([1, QC], F32, tag="rcp")
                nc.vector.reciprocal_approx_fast(out=rcp, in_=x0[0:1, :])
                rcp_b = smallp.tile([CI + 1, QC], F32, tag="rcpb")
                nc.gpsimd.partition_broadcast(rcp_b, rcp)
                x0a = smallp.tile([CI + 1, QC], BF16, tag="x0a")
                nc.vector.tensor_mul(x0a, x0, rcp_b)
                x0a_by_chunk[qi] = x0a
            emit_out(NQC - 1, 0, tail=True)
            emit_out(NQC - 1, 1, tail=True)

    nc.compile()
    return nc


def kernel(x, wf, bf, wg, bg, wh, bh, wv, bv, gamma):
    global _cached_nc, LAST_EXEC_NS
    if _cached_nc is None:
        _cached_nc = _build()
    nc = _cached_nc

    x = np.asarray(x, dtype=np.float32)
    wf = np.asarray(wf, dtype=np.float32)
    bf = np.asarray(bf, dtype=np.float32)
    wg = np.asarray(wg, dtype=np.float32)
    bg = np.asarray(bg, dtype=np.float32)
    wh = np.asarray(wh, dtype=np.float32)
    bh = np.asarray(bh, dtype=np.float32)
    wv = np.asarray(wv, dtype=np.float32)
    bv = np.asarray(bv, dtype=np.float32)
    g0 = float(np.asarray(gamma, dtype=np.float32).reshape(-1)[0])

    bf16 = ml_dtypes.bfloat16
    xf = np.ascontiguousarray(x.reshape(B, C, N))
    # f/g weights replicated 4x along M so f/g land replicated on the
    # four 32-partition strips (enables row-packed logits matmuls).
    wfT = np.ascontiguousarray(np.tile(wf.T, (1, 4))).astype(bf16)
    wgT = np.ascontiguousarray(np.tile(wg.T, (1, 4))).astype(bf16)
    whT = np.ascontiguousarray(wh.T).astype(bf16)
    wvT = np.empty((CI + 1, C), np.float32)              # aug: bias row 0
    wvT[0, :] = g0 * (bv + wv @ bh)
    wvT[1:, :] = g0 * wv.T
    wvT = wvT.astype(bf16)
    wid = np.eye(128, dtype=np.float32).astype(bf16)
    bf4 = np.ascontiguousarray(np.tile(bf, 4).reshape(128, 1))
    bg4 = np.ascontiguousarray(np.tile(bg, 4).reshape(128, 1))

    in_maps = []
    for core in range(NCORES):
        b, half = divmod(core, 2)
        xb = xf[b]
        if half:
            xb = np.concatenate([xb[:, NQ:], xb[:, :NQ]], axis=1)
        in_maps.append(
            {"xbf": np.ascontiguousarray(xb.astype(bf16)), "wfT": wfT,
             "wgT": wgT, "whT": whT, "wvT": wvT, "wid": wid,
             "bf": bf4, "bg": bg4}
        )

    res = run_bass_kernel_spmd(
        nc, in_maps, list(range(NCORES)),
        trace=TRACE or bool(os.environ.get("BASS_KERNEL_TRACE")),
    )
    LAST_EXEC_NS = res.exec_time_ns

    out = np.empty((B, C, N), np.float32)
    for core in range(NCORES):
        b, half = divmod(core, 2)
        out[b][:, half * NQ : (half + 1) * NQ] = res.results[core]["out"]
    return out.reshape(B, C, W, H)


# revision 18
# speedup vs baseline: 1.5350x; 1.2420x over previous
"""Trainium2 Bass kernel for the attention layer:

    f = wf@x+bf; g = wg@x+bg; h = wh@x+bh            (1x1 convs, Ci=32)
    attn = softmax(f^T g, axis=-1)                   (per batch, N=4096)
    out = (wv @ (h @ attn^T) + bv) * gamma + x

Sharding: 8 cores = 4 batches x 2 query-halves (2048 queries each).
Each core receives the full (256, 4096) batch slice with its query half
permuted to the front, so the SPMD program uses fixed offsets.

Per-core dataflow (all-bf16 matmuls, PSUM fp32 accumulate):
  - x arrives bf16 only (2MB); the residual is added from the bf16 copy
    via an IDENTITY MATMUL accumulated into the projection PSUM bank,
    so the output copy is a pure PSUM->SBUF copy (balanceable between
    ACT and DVE) instead of a DVE-only tensor_tensor add.
  - exp is the elementwise bottleneck (32*2048 partition-cycles/core)
    and GPSIMD/DMA cannot touch PSUM, so every PSUM consumer is either
    ACT or DVE: exp groups alternate ACT (true exp, PSUM->bf16) and DVE
    (Schraudolph fast-exp: bf16 bits = int16(l*128/ln2 + 16250) in one
    tensor_scalar, ~3.5% rel err); PSUM->SBUF copies alternate engines;
    softmax divide uses reciprocal_approx_fast.
  - PSUM budget (8 banks): logits pool 3 bufs x 2 banks so the logits
    matmul for group g WAR-waits exp(g-3), letting both exp engines run
    fully parallel; 1 bank for projections/warmup/fillers; 1 bank for
    the x0 accumulator.
  - tiny 1x1 "absorber" matmuls carry the cross-engine semaphore waits
    so the real 512-row matmuls issue back-to-back with their weight
    loads prefetched (an exposed wait blocks LS double-buffering and
    costs ~100ns per matmul).
  - the whole f/g/hT prologue runs before the chunk loop, cycling
    through the logits pool buffers; hT lands 16 key-chunks per PSUM
    bank ([128,16,32] fp32 = 2KB) so one big copy replaces 16 small.
  - per 512-query chunk: 32 row-packed logits matmuls (strip kc%4,
    concurrent PE row bands) -> exp groups of 2 k-chunks -> 32
    x0-accumulation matmuls (hT stationary, ones column in row 0
    accumulating the softmax denominator).
  - projection+residual+DMA for chunk i are DEFERRED into chunk i+1's
    group loop; occasional dummy 512-row matmuls pad PE idle slots so
    the HAM clock gate (K/N pulse gating, 1.2 vs 2.4 GHz) stays open.
"""

import os
import numpy as np
import ml_dtypes

import concourse.bass as bass
import concourse.mybir as mybir
import concourse.tile as tile
from concourse import bacc
from concourse.bass import ts
from concourse.bass_utils import run_bass_kernel_spmd

F32 = mybir.dt.float32
F32R = mybir.dt.float32r
BF16 = mybir.dt.bfloat16
I16 = mybir.dt.int16
EXP = mybir.ActivationFunctionType.Exp
IDENT = mybir.ActivationFunctionType.Identity
MUL = mybir.AluOpType.mult
ADD = mybir.AluOpType.add

B, C, W, H = 4, 256, 64, 64
N = W * H            # 4096 keys/queries per batch
CI = 32              # inner channels
NCORES = 8
NQ = N // 2          # queries per core
QC = 512             # query chunk = one fp32 PSUM bank
NQC = NQ // QC       # 4 query chunks per core
KC = 128             # key chunk = partition dim
NKC = N // KC        # 32 key chunks
GRP = 2              # key chunks per exp group (PSUM banks per tile)
NGRP = NKC // GRP    # 16 groups per chunk
PIPE = 2             # software-pipeline depth (groups) for x0 stage
NWARM = 8            # dummy bf16 matmuls to warm the PE clock gate
FILL_EVERY = 8       # PE filler matmul every this many groups
NSLICE = 4           # x DMA slices

# Schraudolph fast-exp constants (bf16 bits = int16(l*EXP_A + EXP_B))
EXP_A = 128.0 / float(np.log(2.0))
EXP_B = 16250.0
# groups handled by DVE fast-exp (rest go to ACT true exp): 7 of 16
DVE_GROUPS = frozenset({1, 3, 5, 7, 9, 11, 13})

# Trace knob for test harnesses: set kernel.TRACE = True to profile.
TRACE = False
LAST_EXEC_NS = None

_cached_nc = None


def _mm(nc, out, lhsT, rhs, start, stop, tile_position=None):
    nc.tensor.matmul(out, lhsT=lhsT, rhs=rhs, start=start, stop=stop,
                     tile_position=tile_position)


def _build():
    nc = bacc.Bacc(
        "TRN2", target_bir_lowering=False, debug=False, num_devices=NCORES
    )
    xbf_d = nc.dram_tensor("xbf", (C, N), BF16, kind="ExternalInput").ap()
    wfT_d = nc.dram_tensor("wfT", (C, 128), BF16, kind="ExternalInput").ap()
    wgT_d = nc.dram_tensor("wgT", (C, 128), BF16, kind="ExternalInput").ap()
    whT_d = nc.dram_tensor("whT", (C, CI), BF16, kind="ExternalInput").ap()
    wvT_d = nc.dram_tensor("wvT", (CI + 1, C), BF16, kind="ExternalInput").ap()
    wid_d = nc.dram_tensor("wid", (128, 128), BF16, kind="ExternalInput").ap()
    bf_d = nc.dram_tensor("bf", (128, 1), F32, kind="ExternalInput").ap()
    bg_d = nc.dram_tensor("bg", (128, 1), F32, kind="ExternalInput").ap()
    out_d = nc.dram_tensor("out", (C, NQ), F32, kind="ExternalOutput").ap()

    outr = out_d.rearrange("(oc p) n -> p oc n", p=128)
    xbfr = xbf_d.rearrange("(cc p) n -> p cc n", p=128)

    with tile.TileContext(nc) as tc:
        with (
            tc.tile_pool(name="consts", bufs=1) as consts,
            tc.tile_pool(name="data", bufs=1) as data,
            tc.tile_pool(name="eTp", bufs=6) as eTp,
            tc.tile_pool(name="smallp", bufs=2) as smallp,
            tc.tile_pool(name="outp", bufs=3) as outp,
            tc.tile_pool(name="pl", bufs=3, space="PSUM") as pl,
            tc.tile_pool(name="pp", bufs=1, space="PSUM") as pp,
            tc.tile_pool(name="px0", bufs=1, space="PSUM") as px0,
        ):
            # ---- PE + ACT warm-up (overlaps the input DMAs) ----
            scratch = consts.tile([128, QC], BF16)
            nc.vector.memset(scratch, 0.0)

            def fill(n=1):
                # HAM filler: keeps the PE streaming through slots where
                # it would otherwise idle (and re-throttle to 1.2 GHz).
                wps = pp.tile([128, QC], F32, tag="pp", name="wps")
                for _ in range(n):
                    nc.tensor.matmul(
                        wps, lhsT=scratch[:, 0:128], rhs=scratch,
                        start=True, stop=True, skip_group_check=True,
                    )

            fill(NWARM)
            scratch2 = consts.tile([1, 8], F32)
            nc.scalar.activation(out=scratch2, in_=scratch[0:1, 0:8], func=EXP)

            # ---- constants ----
            wfT_sb = consts.tile([128, 2, 128], BF16)
            nc.sync.dma_start(
                out=wfT_sb, in_=wfT_d.rearrange("(cc p) o -> p cc o", p=128)
            )
            wgT_sb = consts.tile([128, 2, 128], BF16)
            nc.sync.dma_start(
                out=wgT_sb, in_=wgT_d.rearrange("(cc p) o -> p cc o", p=128)
            )
            whT_sb = consts.tile([128, 2, CI], BF16)
            nc.sync.dma_start(
                out=whT_sb, in_=whT_d.rearrange("(cc p) o -> p cc o", p=128)
            )
            wvT_sb = consts.tile([CI + 1, 2, 128], BF16)
            nc.sync.dma_start(
                out=wvT_sb, in_=wvT_d.rearrange("p (oc m) -> p oc m", oc=2)
            )
            wid_sb = consts.tile([128, 128], BF16)
            nc.sync.dma_start(out=wid_sb, in_=wid_d)
            bf_sb = consts.tile([128, 1], F32)
            nc.sync.dma_start(out=bf_sb, in_=bf_d)
            bg_sb = consts.tile([128, 1], F32)
            nc.sync.dma_start(out=bg_sb, in_=bg_d)
            ones_sb = consts.tile([128, 1], BF16)
            nc.vector.memset(ones_sb, 1.0)

            # ---- x (bf16, 4 column slices so compute starts early) ----
            xbf_sb = data.tile([128, 2, N], BF16)
            for s in range(NSLICE):
                nc.sync.dma_start(
                    out=xbf_sb[:, :, ts(s, N // NSLICE)],
                    in_=xbfr[:, :, ts(s, N // NSLICE)],
                )

            # ---- f, g (replicated on 4 strips), hT ----
            f_sb = data.tile([128, NQ], BF16)
            g_sb = data.tile([128, N], BF16)
            hT_sb = data.tile([128, NKC, CI + 1], BF16)
            nc.vector.tensor_copy(
                hT_sb[:, :, 0:1], ones_sb.to_broadcast([128, NKC, 1])
            )

            # prologue blocks cycle through the logits pool's 3 buffers;
            # PSUM->SBUF copy work alternates between ACT and DVE.
            def emit_fg(dst, w_sb, b_sb, j, on_act):
                ps = pl.tile([128, GRP, QC], F32, tag="lg", name=f"fg{j}")
                for cc in range(2):
                    _mm(nc, ps[:, 0, :], w_sb[:, cc, :],
                        xbf_sb[:, cc, ts(j, QC)], cc == 0, cc == 1)
                if on_act:
                    nc.scalar.activation(
                        out=dst[:, ts(j, QC)], in_=ps[:, 0, :], func=IDENT,
                        bias=b_sb,
                    )
                else:
                    nc.vector.tensor_scalar_add(
                        dst[:, ts(j, QC)], ps[:, 0, :], b_sb
                    )

            # hT production: one PSUM bank holds 16 key-chunks
            # ([128,16,32] fp32 = 2KB/partition), one big copy each.
            def emit_hT(half, on_act):
                hps = pl.tile([128, 16, CI], F32, tag="lg", name=f"ph{half}")
                for sl in range(16):
                    kc = 16 * half + sl
                    for cc in range(2):
                        _mm(nc, hps[:, sl, :], xbf_sb[:, cc, ts(kc, KC)],
                            whT_sb[:, cc, :], cc == 0, cc == 1)
                dst = hT_sb[:, 16 * half : 16 * (half + 1), 1 : CI + 1]
                if on_act:
                    nc.scalar.copy(dst, hps)
                else:
                    nc.vector.tensor_copy(dst, hps)

            # ---- prologue: f (own queries), g + hT (all keys) ----
            for j in range(4):
                emit_fg(f_sb, wfT_sb, bf_sb, j, on_act=(j % 2 == 0))
                emit_fg(g_sb, wgT_sb, bg_sb, j, on_act=(j % 2 == 1))
                fill(1)
            emit_hT(0, on_act=True)
            for j in range(4, 8):
                emit_fg(g_sb, wgT_sb, bg_sb, j, on_act=(j % 2 == 1))
                fill(1)
            emit_hT(1, on_act=False)

            # deferred projection + residual + output for chunk qi
            x0a_by_chunk = {}

            def emit_out(qi, oc, tail=False):
                x0a = x0a_by_chunk[qi]
                if tail and oc == 1:
                    # final chunk: second projection borrows a logits
                    # bank so both output copies run concurrently.
                    big = pl.tile([128, GRP, QC], F32, tag="lg", name="pst")
                    vps = big[:, 0, :]
                else:
                    vps = pp.tile([128, QC], F32, tag="pp",
                                  name=f"psv{qi}{oc}")
                _mm(nc, vps, wvT_sb[:, oc, :], x0a, True, True)
                # residual fused into the PSUM->SBUF copy (DVE); the PE
                # is the bottleneck so no identity-matmul offload.
                ot = outp.tile([128, QC], F32)
                nc.vector.tensor_add(ot, vps, xbf_sb[:, oc, ts(qi, QC)])
                nc.sync.dma_start(out=outr[:, oc, ts(qi, QC)], in_=ot)

            # ---- main loop over query chunks ----
            for qi in range(NQC):
                # row 0: softmax denominator (ones column in hT);
                # rows 1-32: x0 channels.
                x0 = px0.tile([CI + 1, QC], F32)
                x0q = []
                for gi, g0 in enumerate(range(0, NKC, GRP)):
                    ps = pl.tile([128, GRP, QC], F32, tag="lg")
                    eT = eTp.tile([128, GRP, QC], BF16)
                    for j in range(GRP):
                        kc = g0 + j
                        # row-packed: strip kc%4 holds its own copy of
                        # g/f, so adjacent matmuls execute concurrently
                        # in different PE row bands.
                        s = kc % 4
                        sl = slice(32 * s, 32 * (s + 1))
                        nc.tensor.matmul(
                            ps[:, j, :],
                            lhsT=g_sb[sl, ts(kc, KC)],
                            rhs=f_sb[sl, ts(qi, QC)],
                            start=True, stop=True,
                            tile_position=(32 * s, 0),
                        )
                    if gi in DVE_GROUPS:
                        # Schraudolph fast-exp on DVE: bf16 bits of
                        # exp(l) ~= int16(l*EXP_A + EXP_B)
                        nc.vector.tensor_scalar(
                            out=eT.bitcast(I16), in0=ps,
                            scalar1=EXP_A, scalar2=EXP_B,
                            op0=MUL, op1=ADD,
                        )
                    else:
                        nc.scalar.activation(out=eT, in_=ps, func=EXP)
                    # software-pipeline the x0 stage: its wait on this
                    # group's exp then overlaps later groups' logits in
                    # the in-order PE stream.
                    x0q.append((g0, eT))
                    if len(x0q) > PIPE:
                        pg0, peT = x0q.pop(0)
                        for j in range(GRP):
                            kc = pg0 + j
                            _mm(nc, x0, hT_sb[:, kc, :], peT[:, j, :],
                                kc == 0, kc == NKC - 1)
                    if gi % FILL_EVERY == FILL_EVERY - 1:
                        fill(1)
                    if qi > 0:
                        if gi == 3:
                            emit_out(qi - 1, 0)
                        elif gi == 6:
                            emit_out(qi - 1, 1)
                            del x0a_by_chunk[qi - 1]
                for pg0, peT in x0q:
                    for j in range(GRP):
                        kc = pg0 + j
                        _mm(nc, x0, hT_sb[:, kc, :], peT[:, j, :],
                            kc == 0, kc == NKC - 1)
                if qi == NQC - 1:
                    # keep the HAM window busy through the tail's
                    # reciprocal chain and final projections.
                    fill(3)
                # softmax divide: row 0 of x0 is the denominator
                rcp = smallp.tile([1, QC], F32, tag="rcp")
                nc.vector.reciprocal_approx_fast(out=rcp, in_=x0[0:1, :])
                rcp_b = smallp.tile([CI + 1, QC], F32, tag="rcpb")
                nc.gpsimd.partition_broadcast(rcp_b, rcp)
                x0a = smallp.tile([CI + 1, QC], BF16, tag="x0a")
                nc.vector.tensor_mul(x0a, x0, rcp_b)
                x0a_by_chunk[qi] = x0a
            emit_out(NQC - 1, 0, tail=True)
            emit_out(NQC - 1, 1, tail=True)

    nc.compile()
    return nc


def kernel(x, wf, bf, wg, bg, wh, bh, wv, bv, gamma):
    global _cached_nc, LAST_EXEC_NS
    if _cached_nc is None:
        _cached_nc = _build()
    nc = _cached_nc

    x = np.asarray(x, dtype=np.float32)
    wf = np.asarray(wf, dtype=np.float32)
    bf = np.asarray(bf, dtype=np.float32)
    wg = np.asarray(wg, dtype=np.float32)
    bg = np.asarray(bg, dtype=np.float32)
    wh = np.asarray(wh, dtype=np.float32)
    bh = np.asarray(bh, dtype=np.float32)
    wv = np.asarray(wv, dtype=np.float32)
    bv = np.asarray(bv, dtype=np.float32)
    g0 = float(np.asarray(gamma, dtype=np.float32).reshape(-1)[0])

    bf16 = ml_dtypes.bfloat16
    xf = np.ascontiguousarray(x.reshape(B, C, N))
    # f/g weights replicated 4x along M so f/g land replicated on the
    # four 32-partition strips (enables row-packed logits matmuls).
    wfT = np.ascontiguousarray(np.tile(wf.T, (1, 4))).astype(bf16)
    wgT = np.ascontiguousarray(np.tile(wg.T, (1, 4))).astype(bf16)
    whT = np.ascontiguousarray(wh.T).astype(bf16)
    wvT = np.empty((CI + 1, C), np.float32)              # aug: bias row 0
    wvT[0, :] = g0 * (bv + wv @ bh)
    wvT[1:, :] = g0 * wv.T
    wvT = wvT.astype(bf16)
    wid = np.eye(128, dtype=np.float32).astype(bf16)
    bf4 = np.ascontiguousarray(np.tile(bf, 4).reshape(128, 1))
    bg4 = np.ascontiguousarray(np.tile(bg, 4).reshape(128, 1))

    in_maps = []
    for core in range(NCORES):
        b, half = divmod(core, 2)
        xb = xf[b]
        if half:
            xb = np.concatenate([xb[:, NQ:], xb[:, :NQ]], axis=1)
        in_maps.append(
            {"xbf": np.ascontiguousarray(xb.astype(bf16)), "wfT": wfT,
             "wgT": wgT, "whT": whT, "wvT": wvT, "wid": wid,
             "bf": bf4, "bg": bg4}
        )

    res = run_bass_kernel_spmd(
        nc, in_maps, list(range(NCORES)),
        trace=TRACE or bool(os.environ.get("BASS_KERNEL_TRACE")),
    )
    LAST_EXEC_NS = res.exec_time_ns

    out = np.empty((B, C, N), np.float32)
    for core in range(NCORES):
        b, half = divmod(core, 2)
        out[b][:, half * NQ : (half + 1) * NQ] = res.results[core]["out"]
    return out.reshape(B, C, W, H)


# revision 22
# speedup vs baseline: 1.6307x; 1.0624x over previous
"""Trainium2 Bass kernel for the attention layer:

    f = wf@x+bf; g = wg@x+bg; h = wh@x+bh            (1x1 convs, Ci=32)
    attn = softmax(f^T g, axis=-1)                   (per batch, N=4096)
    out = (wv @ (h @ attn^T) + bv) * gamma + x

Sharding: 8 cores = 4 batches x 2 query-halves (2048 queries each).
Each core receives the full (256, 4096) batch slice with its query half
permuted to the front, so the SPMD program uses fixed offsets.

Per-core dataflow (all-bf16 matmuls, PSUM fp32 accumulate):
  - x arrives bf16 only (2MB); the residual is added from the bf16 copy
    via an IDENTITY MATMUL accumulated into the projection PSUM bank,
    so the output copy is a pure PSUM->SBUF copy (balanceable between
    ACT and DVE) instead of a DVE-only tensor_tensor add.
  - exp is the elementwise bottleneck (32*2048 partition-cycles/core)
    and GPSIMD/DMA cannot touch PSUM, so every PSUM consumer is either
    ACT or DVE: exp groups alternate ACT (true exp, PSUM->bf16) and DVE
    (Schraudolph fast-exp: bf16 bits = int16(l*128/ln2 + 16250) in one
    tensor_scalar, ~3.5% rel err); PSUM->SBUF copies alternate engines;
    softmax divide uses reciprocal_approx_fast.
  - PSUM budget (8 banks): logits pool 3 bufs x 2 banks so the logits
    matmul for group g WAR-waits exp(g-3), letting both exp engines run
    fully parallel; 1 bank for projections/warmup/fillers; 1 bank for
    the x0 accumulator.
  - tiny 1x1 "absorber" matmuls carry the cross-engine semaphore waits
    so the real 512-row matmuls issue back-to-back with their weight
    loads prefetched (an exposed wait blocks LS double-buffering and
    costs ~100ns per matmul).
  - the whole f/g/hT prologue runs before the chunk loop, cycling
    through the logits pool buffers; hT lands 16 key-chunks per PSUM
    bank ([128,16,32] fp32 = 2KB) so one big copy replaces 16 small.
  - per 512-query chunk: 32 row-packed logits matmuls (strip kc%4,
    concurrent PE row bands) -> exp groups of 2 k-chunks -> 32
    x0-accumulation matmuls (hT stationary, ones column in row 0
    accumulating the softmax denominator).
  - projection+residual+DMA for chunk i are DEFERRED into chunk i+1's
    group loop; occasional dummy 512-row matmuls pad PE idle slots so
    the HAM clock gate (K/N pulse gating, 1.2 vs 2.4 GHz) stays open.
"""

import os
import numpy as np
import ml_dtypes

import concourse.bass as bass
import concourse.mybir as mybir
import concourse.tile as tile
from concourse import bacc
from concourse.bass import ts
from concourse.bass_utils import run_bass_kernel_spmd

F32 = mybir.dt.float32
F32R = mybir.dt.float32r
BF16 = mybir.dt.bfloat16
I16 = mybir.dt.int16
EXP = mybir.ActivationFunctionType.Exp
IDENT = mybir.ActivationFunctionType.Identity
MUL = mybir.AluOpType.mult
ADD = mybir.AluOpType.add

B, C, W, H = 4, 256, 64, 64
N = W * H            # 4096 keys/queries per batch
CI = 32              # inner channels
NCORES = 8
NQ = N // 2          # queries per core
QC = 512             # query chunk = one fp32 PSUM bank
NQC = NQ // QC       # 4 query chunks per core
KC = 128             # key chunk = partition dim
NKC = N // KC        # 32 key chunks
GRP = 2              # key chunks per exp group (PSUM banks per tile)
NGRP = NKC // GRP    # 16 groups per chunk
PIPE = 3             # software-pipeline depth (groups) for x0 stage
NWARM = 8            # dummy bf16 matmuls to warm the PE clock gate
FILL_EVERY = 2       # PE filler matmul every this many groups
NSLICE = 4           # x DMA slices

# Schraudolph fast-exp constants (bf16 bits = int16(l*EXP_A + EXP_B))
EXP_A = 128.0 / float(np.log(2.0))
EXP_B = 16250.0
# groups handled by DVE fast-exp (rest go to ACT true exp): 6 of 16
DVE_GROUPS = frozenset({1, 3, 5, 9, 11, 13})

# Trace knob for test harnesses: set kernel.TRACE = True to profile.
TRACE = False
LAST_EXEC_NS = None

_cached_nc = None


def _mm(nc, out, lhsT, rhs, start, stop, tile_position=None):
    nc.tensor.matmul(out, lhsT=lhsT, rhs=rhs, start=start, stop=stop,
                     tile_position=tile_position)


def _build():
    nc = bacc.Bacc(
        "TRN2", target_bir_lowering=False, debug=False, num_devices=NCORES
    )
    xbf_d = nc.dram_tensor("xbf", (C, N), BF16, kind="ExternalInput").ap()
    wfT_d = nc.dram_tensor("wfT", (C, 128), BF16, kind="ExternalInput").ap()
    wgT_d = nc.dram_tensor("wgT", (C, 128), BF16, kind="ExternalInput").ap()
    whT_d = nc.dram_tensor("whT", (C, CI), BF16, kind="ExternalInput").ap()
    wvT_d = nc.dram_tensor("wvT", (CI + 1, C), BF16, kind="ExternalInput").ap()
    wid_d = nc.dram_tensor("wid", (128, 128), BF16, kind="ExternalInput").ap()
    bf_d = nc.dram_tensor("bf", (128, 1), F32, kind="ExternalInput").ap()
    bg_d = nc.dram_tensor("bg", (128, 1), F32, kind="ExternalInput").ap()
    out_d = nc.dram_tensor("out", (C, NQ), F32, kind="ExternalOutput").ap()

    outr = out_d.rearrange("(oc p) n -> p oc n", p=128)
    xbfr = xbf_d.rearrange("(cc p) n -> p cc n", p=128)

    with tile.TileContext(nc) as tc:
        with (
            tc.tile_pool(name="consts", bufs=1) as consts,
            tc.tile_pool(name="data", bufs=1) as data,
            tc.tile_pool(name="eTp", bufs=6) as eTp,
            tc.tile_pool(name="smallp", bufs=2) as smallp,
            tc.tile_pool(name="outp", bufs=3) as outp,
            tc.tile_pool(name="pl", bufs=3, space="PSUM") as pl,
            tc.tile_pool(name="pp", bufs=1, space="PSUM") as pp,
            tc.tile_pool(name="px0", bufs=1, space="PSUM") as px0,
        ):
            # ---- PE + ACT warm-up (overlaps the input DMAs) ----
            scratch = consts.tile([128, QC], BF16)
            nc.vector.memset(scratch, 0.0)

            def fill(n=1, rows=QC):
                # HAM filler: keeps the PE streaming through slots where
                # it would otherwise idle (and re-throttle to 1.2 GHz).
                wps = pp.tile([128, QC], F32, tag="pp", name="wps")
                for _ in range(n):
                    nc.tensor.matmul(
                        wps[:, 0:rows], lhsT=scratch[:, 0:128],
                        rhs=scratch[:, 0:rows],
                        start=True, stop=True, skip_group_check=True,
                    )

            fill(NWARM)
            scratch2 = consts.tile([1, 8], F32)
            nc.scalar.activation(out=scratch2, in_=scratch[0:1, 0:8], func=EXP)

            # ---- constants ----
            wfT_sb = consts.tile([128, 2, 128], BF16)
            nc.sync.dma_start(
                out=wfT_sb, in_=wfT_d.rearrange("(cc p) o -> p cc o", p=128)
            )
            wgT_sb = consts.tile([128, 2, 128], BF16)
            nc.sync.dma_start(
                out=wgT_sb, in_=wgT_d.rearrange("(cc p) o -> p cc o", p=128)
            )
            whT_sb = consts.tile([128, 2, CI], BF16)
            nc.sync.dma_start(
                out=whT_sb, in_=whT_d.rearrange("(cc p) o -> p cc o", p=128)
            )
            wvT_sb = consts.tile([CI + 1, 2, 128], BF16)
            nc.sync.dma_start(
                out=wvT_sb, in_=wvT_d.rearrange("p (oc m) -> p oc m", oc=2)
            )
            wid_sb = consts.tile([128, 128], BF16)
            nc.sync.dma_start(out=wid_sb, in_=wid_d)
            bf_sb = consts.tile([128, 1], F32)
            nc.sync.dma_start(out=bf_sb, in_=bf_d)
            bg_sb = consts.tile([128, 1], F32)
            nc.sync.dma_start(out=bg_sb, in_=bg_d)
            ones_sb = consts.tile([128, 1], BF16)
            nc.vector.memset(ones_sb, 1.0)

            # ---- x (bf16, 4 column slices so compute starts early) ----
            xbf_sb = data.tile([128, 2, N], BF16)
            for s in range(NSLICE):
                nc.sync.dma_start(
                    out=xbf_sb[:, :, ts(s, N // NSLICE)],
                    in_=xbfr[:, :, ts(s, N // NSLICE)],
                )

            # ---- f, g (replicated on 4 strips), hT ----
            f_sb = data.tile([128, NQ], BF16)
            g_sb = data.tile([128, N], BF16)
            hT_sb = data.tile([128, NKC, CI + 1], BF16)
            nc.vector.tensor_copy(
                hT_sb[:, :, 0:1], ones_sb.to_broadcast([128, NKC, 1])
            )

            # prologue blocks cycle through the logits pool's 3 buffers;
            # PSUM->SBUF copy work alternates between ACT and DVE.
            def emit_fg(dst, w_sb, b_sb, j, on_act):
                ps = pl.tile([128, GRP, QC], F32, tag="lg", name=f"fg{j}")
                for cc in range(2):
                    _mm(nc, ps[:, 0, :], w_sb[:, cc, :],
                        xbf_sb[:, cc, ts(j, QC)], cc == 0, cc == 1)
                if on_act:
                    nc.scalar.activation(
                        out=dst[:, ts(j, QC)], in_=ps[:, 0, :], func=IDENT,
                        bias=b_sb,
                    )
                else:
                    nc.vector.tensor_scalar_add(
                        dst[:, ts(j, QC)], ps[:, 0, :], b_sb
                    )

            # hT production: one PSUM bank holds 16 key-chunks
            # ([128,16,32] fp32 = 2KB/partition), one big copy each.
            def emit_hT(half, on_act):
                hps = pl.tile([128, 16, CI], F32, tag="lg", name=f"ph{half}")
                for sl in range(16):
                    kc = 16 * half + sl
                    for cc in range(2):
                        _mm(nc, hps[:, sl, :], xbf_sb[:, cc, ts(kc, KC)],
                            whT_sb[:, cc, :], cc == 0, cc == 1)
                dst = hT_sb[:, 16 * half : 16 * (half + 1), 1 : CI + 1]
                if on_act:
                    nc.scalar.copy(dst, hps)
                else:
                    nc.vector.tensor_copy(dst, hps)

            # ---- prologue: f (own queries), g + hT (all keys) ----
            for j in range(4):
                emit_fg(f_sb, wfT_sb, bf_sb, j, on_act=(j % 2 == 0))
                emit_fg(g_sb, wgT_sb, bg_sb, j, on_act=(j % 2 == 1))
                fill(1)
            emit_hT(0, on_act=True)
            for j in range(4, 8):
                emit_fg(g_sb, wgT_sb, bg_sb, j, on_act=(j % 2 == 1))
                fill(1)
            emit_hT(1, on_act=False)

            # deferred projection + residual + output for chunk qi
            x0a_by_chunk = {}

            def emit_out(qi, oc, tail=False):
                x0a = x0a_by_chunk[qi]
                if tail and oc == 1:
                    # final chunk: second projection borrows a logits
                    # bank so both output copies run concurrently.
                    big = pl.tile([128, GRP, QC], F32, tag="lg", name="pst")
                    vps = big[:, 0, :]
                else:
                    vps = pp.tile([128, QC], F32, tag="pp",
                                  name=f"psv{qi}{oc}")
                _mm(nc, vps, wvT_sb[:, oc, :], x0a, True, True)
                # residual fused into the PSUM->SBUF copy (DVE); the PE
                # is the bottleneck so no identity-matmul offload.
                ot = outp.tile([128, QC], F32)
                nc.vector.tensor_add(ot, vps, xbf_sb[:, oc, ts(qi, QC)])
                nc.sync.dma_start(out=outr[:, oc, ts(qi, QC)], in_=ot)

            # ---- main loop over query chunks ----
            for qi in range(NQC):
                # row 0: softmax denominator (ones column in hT);
                # rows 1-32: x0 channels.
                x0 = px0.tile([CI + 1, QC], F32)
                x0q = []
                for gi, g0 in enumerate(range(0, NKC, GRP)):
                    ps = pl.tile([128, GRP, QC], F32, tag="lg")
                    eT = eTp.tile([128, GRP, QC], BF16)
                    for j in range(GRP):
                        kc = g0 + j
                        # row-packed: strip kc%4 holds its own copy of
                        # g/f, so adjacent matmuls execute concurrently
                        # in different PE row bands.
                        s = kc % 4
                        sl = slice(32 * s, 32 * (s + 1))
                        nc.tensor.matmul(
                            ps[:, j, :],
                            lhsT=g_sb[sl, ts(kc, KC)],
                            rhs=f_sb[sl, ts(qi, QC)],
                            start=True, stop=True,
                            tile_position=(32 * s, 0),
                        )
                    if gi in DVE_GROUPS:
                        # Schraudolph fast-exp on DVE: bf16 bits of
                        # exp(l) ~= int16(l*EXP_A + EXP_B)
                        nc.vector.tensor_scalar(
                            out=eT.bitcast(I16), in0=ps,
                            scalar1=EXP_A, scalar2=EXP_B,
                            op0=MUL, op1=ADD,
                        )
                    else:
                        nc.scalar.activation(out=eT, in_=ps, func=EXP)
                    # software-pipeline the x0 stage: its wait on this
                    # group's exp then overlaps later groups' logits in
                    # the in-order PE stream.
                    x0q.append((g0, eT))
                    if len(x0q) > PIPE:
                        pg0, peT = x0q.pop(0)
                        for j in range(GRP):
                            kc = pg0 + j
                            _mm(nc, x0, hT_sb[:, kc, :], peT[:, j, :],
                                kc == 0, kc == NKC - 1)
                    if gi % FILL_EVERY == FILL_EVERY - 1:
                        fill(1, rows=128)
                    if qi > 0:
                        if gi == 3:
                            emit_out(qi - 1, 0)
                        elif gi == 6:
                            emit_out(qi - 1, 1)
                            del x0a_by_chunk[qi - 1]
                for pg0, peT in x0q:
                    for j in range(GRP):
                        kc = pg0 + j
                        _mm(nc, x0, hT_sb[:, kc, :], peT[:, j, :],
                            kc == 0, kc == NKC - 1)
                if qi == NQC - 1:
                    # keep the HAM window busy through the tail's
                    # reciprocal chain and final projections.
                    fill(3)
                # softmax divide: row 0 of x0 is the denominator
                rcp = smallp.tile([1, QC], F32, tag="rcp")
                nc.vector.reciprocal_approx_fast(out=rcp, in_=x0[0:1, :])
                rcp_b = smallp.tile([CI + 1, QC], F32, tag="rcpb")
                nc.gpsimd.partition_broadcast(rcp_b, rcp)
                x0a = smallp.tile([CI + 1, QC], BF16, tag="x0a")
                nc.vector.tensor_mul(x0a, x0, rcp_b)
                x0a_by_chunk[qi] = x0a
            emit_out(NQC - 1, 0, tail=True)
            emit_out(NQC - 1, 1, tail=True)

    nc.compile()
    return nc


def kernel(x, wf, bf, wg, bg, wh, bh, wv, bv, gamma):
    global _cached_nc, LAST_EXEC_NS
    if _cached_nc is None:
        _cached_nc = _build()
    nc = _cached_nc

    x = np.asarray(x, dtype=np.float32)
    wf = np.asarray(wf, dtype=np.float32)
    bf = np.asarray(bf, dtype=np.float32)
    wg = np.asarray(wg, dtype=np.float32)
    bg = np.asarray(bg, dtype=np.float32)
    wh = np.asarray(wh, dtype=np.float32)
    bh = np.asarray(bh, dtype=np.float32)
    wv = np.asarray(wv, dtype=np.float32)
    bv = np.asarray(bv, dtype=np.float32)
    g0 = float(np.asarray(gamma, dtype=np.float32).reshape(-1)[0])

    bf16 = ml_dtypes.bfloat16
    xf = np.ascontiguousarray(x.reshape(B, C, N))
    # f/g weights replicated 4x along M so f/g land replicated on the
    # four 32-partition strips (enables row-packed logits matmuls).
    wfT = np.ascontiguousarray(np.tile(wf.T, (1, 4))).astype(bf16)
    wgT = np.ascontiguousarray(np.tile(wg.T, (1, 4))).astype(bf16)
    whT = np.ascontiguousarray(wh.T).astype(bf16)
    wvT = np.empty((CI + 1, C), np.float32)              # aug: bias row 0
    wvT[0, :] = g0 * (bv + wv @ bh)
    wvT[1:, :] = g0 * wv.T
    wvT = wvT.astype(bf16)
    wid = np.eye(128, dtype=np.float32).astype(bf16)
    bf4 = np.ascontiguousarray(np.tile(bf, 4).reshape(128, 1))
    bg4 = np.ascontiguousarray(np.tile(bg, 4).reshape(128, 1))

    in_maps = []
    for core in range(NCORES):
        b, half = divmod(core, 2)
        xb = xf[b]
        if half:
            xb = np.concatenate([xb[:, NQ:], xb[:, :NQ]], axis=1)
        in_maps.append(
            {"xbf": np.ascontiguousarray(xb.astype(bf16)), "wfT": wfT,
             "wgT": wgT, "whT": whT, "wvT": wvT, "wid": wid,
             "bf": bf4, "bg": bg4}
        )

    res = run_bass_kernel_spmd(
        nc, in_maps, list(range(NCORES)),
        trace=TRACE or bool(os.environ.get("BASS_KERNEL_TRACE")),
    )
    LAST_EXEC_NS = res.exec_time_ns

    out = np.empty((B, C, N), np.float32)
    for core in range(NCORES):
        b, half = divmod(core, 2)
        out[b][:, half * NQ : (half + 1) * NQ] = res.results[core]["out"]
    return out.reshape(B, C, W, H)


# revision 23
# speedup vs baseline: 1.6443x; 1.0083x over previous
"""Trainium2 Bass kernel for the attention layer:

    f = wf@x+bf; g = wg@x+bg; h = wh@x+bh            (1x1 convs, Ci=32)
    attn = softmax(f^T g, axis=-1)                   (per batch, N=4096)
    out = (wv @ (h @ attn^T) + bv) * gamma + x

Sharding: 8 cores = 4 batches x 2 query-halves (2048 queries each).
Each core receives the full (256, 4096) batch slice with its query half
permuted to the front, so the SPMD program uses fixed offsets.

Per-core dataflow (all-bf16 matmuls, PSUM fp32 accumulate):
  - x arrives bf16 only (2MB); the residual is added from the bf16 copy
    via an IDENTITY MATMUL accumulated into the projection PSUM bank,
    so the output copy is a pure PSUM->SBUF copy (balanceable between
    ACT and DVE) instead of a DVE-only tensor_tensor add.
  - exp is the elementwise bottleneck (32*2048 partition-cycles/core)
    and GPSIMD/DMA cannot touch PSUM, so every PSUM consumer is either
    ACT or DVE: exp groups alternate ACT (true exp, PSUM->bf16) and DVE
    (Schraudolph fast-exp: bf16 bits = int16(l*128/ln2 + 16250) in one
    tensor_scalar, ~3.5% rel err); PSUM->SBUF copies alternate engines;
    softmax divide uses reciprocal_approx_fast.
  - PSUM budget (8 banks): logits pool 3 bufs x 2 banks so the logits
    matmul for group g WAR-waits exp(g-3), letting both exp engines run
    fully parallel; 1 bank for projections/warmup/fillers; 1 bank for
    the x0 accumulator.
  - tiny 1x1 "absorber" matmuls carry the cross-engine semaphore waits
    so the real 512-row matmuls issue back-to-back with their weight
    loads prefetched (an exposed wait blocks LS double-buffering and
    costs ~100ns per matmul).
  - the whole f/g/hT prologue runs before the chunk loop, cycling
    through the logits pool buffers; hT lands 16 key-chunks per PSUM
    bank ([128,16,32] fp32 = 2KB) so one big copy replaces 16 small.
  - per 512-query chunk: 32 row-packed logits matmuls (strip kc%4,
    concurrent PE row bands) -> exp groups of 2 k-chunks -> 32
    x0-accumulation matmuls (hT stationary, ones column in row 0
    accumulating the softmax denominator).
  - projection+residual+DMA for chunk i are DEFERRED into chunk i+1's
    group loop; occasional dummy 512-row matmuls pad PE idle slots so
    the HAM clock gate (K/N pulse gating, 1.2 vs 2.4 GHz) stays open.
"""

import os
import numpy as np
import ml_dtypes

import concourse.bass as bass
import concourse.mybir as mybir
import concourse.tile as tile
from concourse import bacc
from concourse.bass import ts
from concourse.bass_utils import run_bass_kernel_spmd

F32 = mybir.dt.float32
F32R = mybir.dt.float32r
BF16 = mybir.dt.bfloat16
I16 = mybir.dt.int16
EXP = mybir.ActivationFunctionType.Exp
IDENT = mybir.ActivationFunctionType.Identity
MUL = mybir.AluOpType.mult
ADD = mybir.AluOpType.add

B, C, W, H = 4, 256, 64, 64
N = W * H            # 4096 keys/queries per batch
CI = 32              # inner channels
NCORES = 8
NQ = N // 2          # queries per core
QC = 512             # query chunk = one fp32 PSUM bank
NQC = NQ // QC       # 4 query chunks per core
KC = 128             # key chunk = partition dim
NKC = N // KC        # 32 key chunks
GRP = 2              # key chunks per exp group (PSUM banks per tile)
NGRP = NKC // GRP    # 16 groups per chunk
PIPE = 3             # software-pipeline depth (groups) for x0 stage
NWARM = 8            # dummy bf16 matmuls to warm the PE clock gate
FILL_EVERY = 2       # PE filler matmul every this many groups
NSLICE = 8           # x DMA slices

# Schraudolph fast-exp constants (bf16 bits = int16(l*EXP_A + EXP_B))
EXP_A = 128.0 / float(np.log(2.0))
EXP_B = 16250.0
# groups handled by DVE fast-exp (rest go to ACT true exp): 6 of 16
DVE_GROUPS = frozenset({1, 3, 5, 9, 11, 13})

# Trace knob for test harnesses: set kernel.TRACE = True to profile.
TRACE = False
LAST_EXEC_NS = None

_cached_nc = None


def _mm(nc, out, lhsT, rhs, start, stop, tile_position=None):
    nc.tensor.matmul(out, lhsT=lhsT, rhs=rhs, start=start, stop=stop,
                     tile_position=tile_position)


def _build():
    nc = bacc.Bacc(
        "TRN2", target_bir_lowering=False, debug=False, num_devices=NCORES
    )
    xbf_d = nc.dram_tensor("xbf", (C, N), BF16, kind="ExternalInput").ap()
    wfT_d = nc.dram_tensor("wfT", (C, 128), BF16, kind="ExternalInput").ap()
    wgT_d = nc.dram_tensor("wgT", (C, 128), BF16, kind="ExternalInput").ap()
    whT_d = nc.dram_tensor("whT", (C, CI), BF16, kind="ExternalInput").ap()
    wvT_d = nc.dram_tensor("wvT", (CI + 1, C), BF16, kind="ExternalInput").ap()
    wid_d = nc.dram_tensor("wid", (128, 128), BF16, kind="ExternalInput").ap()
    bf_d = nc.dram_tensor("bf", (128, 1), F32, kind="ExternalInput").ap()
    bg_d = nc.dram_tensor("bg", (128, 1), F32, kind="ExternalInput").ap()
    out_d = nc.dram_tensor("out", (C, NQ), F32, kind="ExternalOutput").ap()

    outr = out_d.rearrange("(oc p) n -> p oc n", p=128)
    xbfr = xbf_d.rearrange("(cc p) n -> p cc n", p=128)

    with tile.TileContext(nc) as tc:
        with (
            tc.tile_pool(name="consts", bufs=1) as consts,
            tc.tile_pool(name="data", bufs=1) as data,
            tc.tile_pool(name="eTp", bufs=6) as eTp,
            tc.tile_pool(name="smallp", bufs=2) as smallp,
            tc.tile_pool(name="outp", bufs=3) as outp,
            tc.tile_pool(name="pl", bufs=3, space="PSUM") as pl,
            tc.tile_pool(name="pp", bufs=1, space="PSUM") as pp,
            tc.tile_pool(name="px0", bufs=1, space="PSUM") as px0,
        ):
            # ---- PE + ACT warm-up (overlaps the input DMAs) ----
            scratch = consts.tile([128, QC], BF16)
            nc.vector.memset(scratch, 0.0)

            def fill(n=1, rows=QC):
                # HAM filler: keeps the PE streaming through slots where
                # it would otherwise idle (and re-throttle to 1.2 GHz).
                wps = pp.tile([128, QC], F32, tag="pp", name="wps")
                for _ in range(n):
                    nc.tensor.matmul(
                        wps[:, 0:rows], lhsT=scratch[:, 0:128],
                        rhs=scratch[:, 0:rows],
                        start=True, stop=True, skip_group_check=True,
                    )

            fill(NWARM)
            scratch2 = consts.tile([1, 8], F32)
            nc.scalar.activation(out=scratch2, in_=scratch[0:1, 0:8], func=EXP)

            # ---- constants ----
            wfT_sb = consts.tile([128, 2, 128], BF16)
            nc.sync.dma_start(
                out=wfT_sb, in_=wfT_d.rearrange("(cc p) o -> p cc o", p=128)
            )
            wgT_sb = consts.tile([128, 2, 128], BF16)
            nc.sync.dma_start(
                out=wgT_sb, in_=wgT_d.rearrange("(cc p) o -> p cc o", p=128)
            )
            whT_sb = consts.tile([128, 2, CI], BF16)
            nc.sync.dma_start(
                out=whT_sb, in_=whT_d.rearrange("(cc p) o -> p cc o", p=128)
            )
            wvT_sb = consts.tile([CI + 1, 2, 128], BF16)
            nc.sync.dma_start(
                out=wvT_sb, in_=wvT_d.rearrange("p (oc m) -> p oc m", oc=2)
            )
            wid_sb = consts.tile([128, 128], BF16)
            nc.sync.dma_start(out=wid_sb, in_=wid_d)
            bf_sb = consts.tile([128, 1], F32)
            nc.sync.dma_start(out=bf_sb, in_=bf_d)
            bg_sb = consts.tile([128, 1], F32)
            nc.sync.dma_start(out=bg_sb, in_=bg_d)
            ones_sb = consts.tile([128, 1], BF16)
            nc.vector.memset(ones_sb, 1.0)

            # ---- x (bf16, 4 column slices so compute starts early) ----
            xbf_sb = data.tile([128, 2, N], BF16)
            for s in range(NSLICE):
                nc.sync.dma_start(
                    out=xbf_sb[:, :, ts(s, N // NSLICE)],
                    in_=xbfr[:, :, ts(s, N // NSLICE)],
                )

            # ---- f, g (replicated on 4 strips), hT ----
            f_sb = data.tile([128, NQ], BF16)
            g_sb = data.tile([128, N], BF16)
            hT_sb = data.tile([128, NKC, CI + 1], BF16)
            nc.vector.tensor_copy(
                hT_sb[:, :, 0:1], ones_sb.to_broadcast([128, NKC, 1])
            )

            # prologue blocks cycle through the logits pool's 3 buffers;
            # PSUM->SBUF copy work alternates between ACT and DVE.
            def emit_fg(dst, w_sb, b_sb, j, on_act):
                ps = pl.tile([128, GRP, QC], F32, tag="lg", name=f"fg{j}")
                for cc in range(2):
                    _mm(nc, ps[:, 0, :], w_sb[:, cc, :],
                        xbf_sb[:, cc, ts(j, QC)], cc == 0, cc == 1)
                if on_act:
                    nc.scalar.activation(
                        out=dst[:, ts(j, QC)], in_=ps[:, 0, :], func=IDENT,
                        bias=b_sb,
                    )
                else:
                    nc.vector.tensor_scalar_add(
                        dst[:, ts(j, QC)], ps[:, 0, :], b_sb
                    )

            # hT production: one PSUM bank holds 16 key-chunks
            # ([128,16,32] fp32 = 2KB/partition), one big copy each.
            def emit_hT(half, on_act):
                hps = pl.tile([128, 16, CI], F32, tag="lg", name=f"ph{half}")
                for sl in range(16):
                    kc = 16 * half + sl
                    for cc in range(2):
                        _mm(nc, hps[:, sl, :], xbf_sb[:, cc, ts(kc, KC)],
                            whT_sb[:, cc, :], cc == 0, cc == 1)
                dst = hT_sb[:, 16 * half : 16 * (half + 1), 1 : CI + 1]
                if on_act:
                    nc.scalar.copy(dst, hps)
                else:
                    nc.vector.tensor_copy(dst, hps)

            # ---- prologue: f (own queries), g + hT (all keys) ----
            for j in range(4):
                emit_fg(f_sb, wfT_sb, bf_sb, j, on_act=(j % 2 == 0))
                emit_fg(g_sb, wgT_sb, bg_sb, j, on_act=(j % 2 == 1))
                fill(1)
            emit_hT(0, on_act=True)
            for j in range(4, 8):
                emit_fg(g_sb, wgT_sb, bg_sb, j, on_act=(j % 2 == 1))
                fill(1)
            emit_hT(1, on_act=False)

            # deferred projection + residual + output for chunk qi
            x0a_by_chunk = {}

            def emit_out(qi, oc, tail=False):
                x0a = x0a_by_chunk[qi]
                if tail and oc == 1:
                    # final chunk: second projection borrows a logits
                    # bank so both output copies run concurrently.
                    big = pl.tile([128, GRP, QC], F32, tag="lg", name="pst")
                    vps = big[:, 0, :]
                else:
                    vps = pp.tile([128, QC], F32, tag="pp",
                                  name=f"psv{qi}{oc}")
                _mm(nc, vps, wvT_sb[:, oc, :], x0a, True, True)
                # residual fused into the PSUM->SBUF copy (DVE); the PE
                # is the bottleneck so no identity-matmul offload.
                ot = outp.tile([128, QC], F32)
                nc.vector.tensor_add(ot, vps, xbf_sb[:, oc, ts(qi, QC)])
                nc.sync.dma_start(out=outr[:, oc, ts(qi, QC)], in_=ot)

            # ---- main loop over query chunks ----
            for qi in range(NQC):
                # row 0: softmax denominator (ones column in hT);
                # rows 1-32: x0 channels.
                x0 = px0.tile([CI + 1, QC], F32)
                x0q = []
                for gi, g0 in enumerate(range(0, NKC, GRP)):
                    ps = pl.tile([128, GRP, QC], F32, tag="lg")
                    eT = eTp.tile([128, GRP, QC], BF16)
                    for j in range(GRP):
                        kc = g0 + j
                        # row-packed: strip kc%4 holds its own copy of
                        # g/f, so adjacent matmuls execute concurrently
                        # in different PE row bands.
                        s = kc % 4
                        sl = slice(32 * s, 32 * (s + 1))
                        nc.tensor.matmul(
                            ps[:, j, :],
                            lhsT=g_sb[sl, ts(kc, KC)],
                            rhs=f_sb[sl, ts(qi, QC)],
                            start=True, stop=True,
                            tile_position=(32 * s, 0),
                        )
                    if gi in DVE_GROUPS:
                        # Schraudolph fast-exp on DVE: bf16 bits of
                        # exp(l) ~= int16(l*EXP_A + EXP_B)
                        nc.vector.tensor_scalar(
                            out=eT.bitcast(I16), in0=ps,
                            scalar1=EXP_A, scalar2=EXP_B,
                            op0=MUL, op1=ADD,
                        )
                    else:
                        nc.scalar.activation(out=eT, in_=ps, func=EXP)
                    # software-pipeline the x0 stage: its wait on this
                    # group's exp then overlaps later groups' logits in
                    # the in-order PE stream.
                    x0q.append((g0, eT))
                    if len(x0q) > PIPE:
                        pg0, peT = x0q.pop(0)
                        for j in range(GRP):
                            kc = pg0 + j
                            _mm(nc, x0, hT_sb[:, kc, :], peT[:, j, :],
                                kc == 0, kc == NKC - 1)
                    if gi % FILL_EVERY == FILL_EVERY - 1:
                        fill(1, rows=128)
                    if qi > 0:
                        if gi == 3:
                            emit_out(qi - 1, 0)
                        elif gi == 6:
                            emit_out(qi - 1, 1)
                            del x0a_by_chunk[qi - 1]
                for pg0, peT in x0q:
                    for j in range(GRP):
                        kc = pg0 + j
                        _mm(nc, x0, hT_sb[:, kc, :], peT[:, j, :],
                            kc == 0, kc == NKC - 1)
                if qi == NQC - 1:
                    # keep the HAM window busy through the tail's
                    # reciprocal chain and final projections.
                    fill(3)
                # softmax divide: row 0 of x0 is the denominator
                rcp = smallp.tile([1, QC], F32, tag="rcp")
                nc.vector.reciprocal_approx_fast(out=rcp, in_=x0[0:1, :])
                rcp_b = smallp.tile([CI + 1, QC], F32, tag="rcpb")
                nc.gpsimd.partition_broadcast(rcp_b, rcp)
                x0a = smallp.tile([CI + 1, QC], BF16, tag="x0a")
                nc.vector.tensor_mul(x0a, x0, rcp_b)
                x0a_by_chunk[qi] = x0a
            emit_out(NQC - 1, 0, tail=True)
            emit_out(NQC - 1, 1, tail=True)

    nc.compile()
    return nc


def kernel(x, wf, bf, wg, bg, wh, bh, wv, bv, gamma):
    global _cached_nc, LAST_EXEC_NS
    if _cached_nc is None:
        _cached_nc = _build()
    nc = _cached_nc

    x = np.asarray(x, dtype=np.float32)
    wf = np.asarray(wf, dtype=np.float32)
    bf = np.asarray(bf, dtype=np.float32)
    wg = np.asarray(wg, dtype=np.float32)
    bg = np.asarray(bg, dtype=np.float32)
    wh = np.asarray(wh, dtype=np.float32)
    bh = np.asarray(bh, dtype=np.float32)
    wv = np.asarray(wv, dtype=np.float32)
    bv = np.asarray(bv, dtype=np.float32)
    g0 = float(np.asarray(gamma, dtype=np.float32).reshape(-1)[0])

    bf16 = ml_dtypes.bfloat16
    xf = np.ascontiguousarray(x.reshape(B, C, N))
    # f/g weights replicated 4x along M so f/g land replicated on the
    # four 32-partition strips (enables row-packed logits matmuls).
    wfT = np.ascontiguousarray(np.tile(wf.T, (1, 4))).astype(bf16)
    wgT = np.ascontiguousarray(np.tile(wg.T, (1, 4))).astype(bf16)
    whT = np.ascontiguousarray(wh.T).astype(bf16)
    wvT = np.empty((CI + 1, C), np.float32)              # aug: bias row 0
    wvT[0, :] = g0 * (bv + wv @ bh)
    wvT[1:, :] = g0 * wv.T
    wvT = wvT.astype(bf16)
    wid = np.eye(128, dtype=np.float32).astype(bf16)
    bf4 = np.ascontiguousarray(np.tile(bf, 4).reshape(128, 1))
    bg4 = np.ascontiguousarray(np.tile(bg, 4).reshape(128, 1))

    in_maps = []
    for core in range(NCORES):
        b, half = divmod(core, 2)
        xb = xf[b]
        if half:
            xb = np.concatenate([xb[:, NQ:], xb[:, :NQ]], axis=1)
        in_maps.append(
            {"xbf": np.ascontiguousarray(xb.astype(bf16)), "wfT": wfT,
             "wgT": wgT, "whT": whT, "wvT": wvT, "wid": wid,
             "bf": bf4, "bg": bg4}
        )

    res = run_bass_kernel_spmd(
        nc, in_maps, list(range(NCORES)),
        trace=TRACE or bool(os.environ.get("BASS_KERNEL_TRACE")),
    )
    LAST_EXEC_NS = res.exec_time_ns

    out = np.empty((B, C, N), np.float32)
    for core in range(NCORES):
        b, half = divmod(core, 2)
        out[b][:, half * NQ : (half + 1) * NQ] = res.results[core]["out"]
    return out.reshape(B, C, W, H)


# revision 25
# speedup vs baseline: 1.6700x; 1.0156x over previous
"""Trainium2 Bass kernel for the attention layer:

    f = wf@x+bf; g = wg@x+bg; h = wh@x+bh            (1x1 convs, Ci=32)
    attn = softmax(f^T g, axis=-1)                   (per batch, N=4096)
    out = (wv @ (h @ attn^T) + bv) * gamma + x

Sharding: 8 cores = 4 batches x 2 query-halves (2048 queries each).
Each core receives the full (256, 4096) batch slice with its query half
permuted to the front, so the SPMD program uses fixed offsets.

Per-core dataflow (all-bf16 matmuls, PSUM fp32 accumulate):
  - x arrives bf16 only (2MB); the residual is added from the bf16 copy
    via an IDENTITY MATMUL accumulated into the projection PSUM bank,
    so the output copy is a pure PSUM->SBUF copy (balanceable between
    ACT and DVE) instead of a DVE-only tensor_tensor add.
  - exp is the elementwise bottleneck (32*2048 partition-cycles/core)
    and GPSIMD/DMA cannot touch PSUM, so every PSUM consumer is either
    ACT or DVE: exp groups alternate ACT (true exp, PSUM->bf16) and DVE
    (Schraudolph fast-exp: bf16 bits = int16(l*128/ln2 + 16250) in one
    tensor_scalar, ~3.5% rel err); PSUM->SBUF copies alternate engines;
    softmax divide uses reciprocal_approx_fast.
  - PSUM budget (8 banks): logits pool 3 bufs x 2 banks so the logits
    matmul for group g WAR-waits exp(g-3), letting both exp engines run
    fully parallel; 1 bank for projections/warmup/fillers; 1 bank for
    the x0 accumulator.
  - tiny 1x1 "absorber" matmuls carry the cross-engine semaphore waits
    so the real 512-row matmuls issue back-to-back with their weight
    loads prefetched (an exposed wait blocks LS double-buffering and
    costs ~100ns per matmul).
  - the whole f/g/hT prologue runs before the chunk loop, cycling
    through the logits pool buffers; hT lands 16 key-chunks per PSUM
    bank ([128,16,32] fp32 = 2KB) so one big copy replaces 16 small.
  - per 512-query chunk: 32 row-packed logits matmuls (strip kc%4,
    concurrent PE row bands) -> exp groups of 2 k-chunks -> 32
    x0-accumulation matmuls (hT stationary, ones column in row 0
    accumulating the softmax denominator).
  - projection+residual+DMA for chunk i are DEFERRED into chunk i+1's
    group loop; occasional dummy 512-row matmuls pad PE idle slots so
    the HAM clock gate (K/N pulse gating, 1.2 vs 2.4 GHz) stays open.
"""

import os
import numpy as np
import ml_dtypes

import concourse.bass as bass
import concourse.mybir as mybir
import concourse.tile as tile
from concourse import bacc
from concourse.bass import ts
from concourse.bass_utils import run_bass_kernel_spmd

F32 = mybir.dt.float32
F32R = mybir.dt.float32r
BF16 = mybir.dt.bfloat16
I16 = mybir.dt.int16
EXP = mybir.ActivationFunctionType.Exp
IDENT = mybir.ActivationFunctionType.Identity
MUL = mybir.AluOpType.mult
ADD = mybir.AluOpType.add

B, C, W, H = 4, 256, 64, 64
N = W * H            # 4096 keys/queries per batch
CI = 32              # inner channels
NCORES = 8
NQ = N // 2          # queries per core
QC = 512             # query chunk = one fp32 PSUM bank
NQC = NQ // QC       # 4 query chunks per core
KC = 128             # key chunk = partition dim
NKC = N // KC        # 32 key chunks
GRP = 2              # key chunks per exp group (PSUM banks per tile)
NGRP = NKC // GRP    # 16 groups per chunk
PIPE = 3             # software-pipeline depth (groups) for x0 stage
NWARM = 8            # dummy bf16 matmuls to warm the PE clock gate
FILL_EVERY = 2       # PE filler matmul every this many groups
NSLICE = 8           # x DMA slices

# Schraudolph fast-exp constants (bf16 bits = int16(l*EXP_A + EXP_B))
EXP_A = 128.0 / float(np.log(2.0))
EXP_B = 16250.0
# groups handled by DVE fast-exp (rest go to ACT true exp): 6 of 16
DVE_GROUPS = frozenset({1, 3, 5, 9, 11, 13})

# Trace knob for test harnesses: set kernel.TRACE = True to profile.
TRACE = False
LAST_EXEC_NS = None

_cached_nc = None


def _mm(nc, out, lhsT, rhs, start, stop, tile_position=None):
    nc.tensor.matmul(out, lhsT=lhsT, rhs=rhs, start=start, stop=stop,
                     tile_position=tile_position)


def _build():
    nc = bacc.Bacc(
        "TRN2", target_bir_lowering=False, debug=False, num_devices=NCORES
    )
    xbf_d = nc.dram_tensor("xbf", (C, N), BF16, kind="ExternalInput").ap()
    wfT_d = nc.dram_tensor("wfT", (C, 128), BF16, kind="ExternalInput").ap()
    wgT_d = nc.dram_tensor("wgT", (C, 128), BF16, kind="ExternalInput").ap()
    whT_d = nc.dram_tensor("whT", (C, CI), BF16, kind="ExternalInput").ap()
    wvT_d = nc.dram_tensor("wvT", (CI + 1, C), BF16, kind="ExternalInput").ap()
    wid_d = nc.dram_tensor("wid", (128, 128), BF16, kind="ExternalInput").ap()
    bf_d = nc.dram_tensor("bf", (128, 1), F32, kind="ExternalInput").ap()
    bg_d = nc.dram_tensor("bg", (128, 1), F32, kind="ExternalInput").ap()
    out_d = nc.dram_tensor("out", (C, NQ), F32, kind="ExternalOutput").ap()

    outr = out_d.rearrange("(oc p) n -> p oc n", p=128)
    xbfr = xbf_d.rearrange("(cc p) n -> p cc n", p=128)

    with tile.TileContext(nc) as tc:
        with (
            tc.tile_pool(name="consts", bufs=1) as consts,
            tc.tile_pool(name="data", bufs=1) as data,
            tc.tile_pool(name="eTp", bufs=6) as eTp,
            tc.tile_pool(name="smallp", bufs=2) as smallp,
            tc.tile_pool(name="outp", bufs=3) as outp,
            tc.tile_pool(name="pl", bufs=3, space="PSUM") as pl,
            tc.tile_pool(name="pp", bufs=1, space="PSUM") as pp,
            tc.tile_pool(name="px0", bufs=1, space="PSUM") as px0,
        ):
            # ---- PE + ACT warm-up (overlaps the input DMAs) ----
            scratch = consts.tile([128, QC], BF16)
            nc.vector.memset(scratch, 0.0)

            def fill(n=1, rows=QC):
                # HAM filler: keeps the PE streaming through slots where
                # it would otherwise idle (and re-throttle to 1.2 GHz).
                wps = pp.tile([128, QC], F32, tag="pp", name="wps")
                for _ in range(n):
                    nc.tensor.matmul(
                        wps[:, 0:rows], lhsT=scratch[:, 0:128],
                        rhs=scratch[:, 0:rows],
                        start=True, stop=True, skip_group_check=True,
                    )

            fill(NWARM)
            scratch2 = consts.tile([1, 8], F32)
            nc.scalar.activation(out=scratch2, in_=scratch[0:1, 0:8], func=EXP)

            # ---- constants ----
            wfT_sb = consts.tile([128, 2, 128], BF16)
            nc.sync.dma_start(
                out=wfT_sb, in_=wfT_d.rearrange("(cc p) o -> p cc o", p=128)
            )
            wgT_sb = consts.tile([128, 2, 128], BF16)
            nc.sync.dma_start(
                out=wgT_sb, in_=wgT_d.rearrange("(cc p) o -> p cc o", p=128)
            )
            whT_sb = consts.tile([128, 2, CI], BF16)
            nc.sync.dma_start(
                out=whT_sb, in_=whT_d.rearrange("(cc p) o -> p cc o", p=128)
            )
            wvT_sb = consts.tile([CI + 1, 2, 128], BF16)
            nc.sync.dma_start(
                out=wvT_sb, in_=wvT_d.rearrange("p (oc m) -> p oc m", oc=2)
            )
            wid_sb = consts.tile([128, 128], BF16)
            nc.sync.dma_start(out=wid_sb, in_=wid_d)
            bf_sb = consts.tile([128, 1], F32)
            nc.sync.dma_start(out=bf_sb, in_=bf_d)
            bg_sb = consts.tile([128, 1], F32)
            nc.sync.dma_start(out=bg_sb, in_=bg_d)
            ones_sb = consts.tile([128, 1], BF16)
            nc.vector.memset(ones_sb, 1.0)

            # ---- x (bf16, 4 column slices so compute starts early) ----
            xbf_sb = data.tile([128, 2, N], BF16)
            for s in range(NSLICE):
                nc.sync.dma_start(
                    out=xbf_sb[:, :, ts(s, N // NSLICE)],
                    in_=xbfr[:, :, ts(s, N // NSLICE)],
                )

            # ---- f, g (replicated on 4 strips), hT ----
            f_sb = data.tile([128, NQ], BF16)
            g_sb = data.tile([128, N], BF16)
            hT_sb = data.tile([128, NKC, CI + 1], BF16)
            nc.vector.tensor_copy(
                hT_sb[:, :, 0:1], ones_sb.to_broadcast([128, NKC, 1])
            )

            # prologue blocks cycle through the logits pool's 3 buffers;
            # PSUM->SBUF copy work alternates between ACT and DVE.
            def emit_fg(dst, w_sb, b_sb, j, on_act):
                ps = pl.tile([128, GRP, QC], F32, tag="lg", name=f"fg{j}")
                for cc in range(2):
                    _mm(nc, ps[:, 0, :], w_sb[:, cc, :],
                        xbf_sb[:, cc, ts(j, QC)], cc == 0, cc == 1)
                if on_act:
                    nc.scalar.activation(
                        out=dst[:, ts(j, QC)], in_=ps[:, 0, :], func=IDENT,
                        bias=b_sb,
                    )
                else:
                    nc.vector.tensor_scalar_add(
                        dst[:, ts(j, QC)], ps[:, 0, :], b_sb
                    )

            # hT production: one PSUM bank holds 16 key-chunks
            # ([128,16,32] fp32 = 2KB/partition), one big copy each.
            def emit_hT(half, on_act):
                hps = pl.tile([128, 16, CI], F32, tag="lg", name=f"ph{half}")
                for sl in range(16):
                    kc = 16 * half + sl
                    for cc in range(2):
                        _mm(nc, hps[:, sl, :], xbf_sb[:, cc, ts(kc, KC)],
                            whT_sb[:, cc, :], cc == 0, cc == 1)
                dst = hT_sb[:, 16 * half : 16 * (half + 1), 1 : CI + 1]
                if on_act:
                    nc.scalar.copy(dst, hps)
                else:
                    nc.vector.tensor_copy(dst, hps)

            # ---- prologue: f (own queries), g + hT (all keys) ----
            for j in range(4):
                emit_fg(f_sb, wfT_sb, bf_sb, j, on_act=(j % 2 == 0))
                emit_fg(g_sb, wgT_sb, bg_sb, j, on_act=(j % 2 == 1))
                fill(1)
            emit_hT(0, on_act=True)
            for j in range(4, 8):
                emit_fg(g_sb, wgT_sb, bg_sb, j, on_act=(j % 2 == 1))
                fill(1)
            emit_hT(1, on_act=False)

            # deferred projection + residual + output for chunk qi
            x0a_by_chunk = {}

            def emit_out(qi, oc, tail=False):
                x0a = x0a_by_chunk[qi]
                if tail and oc == 1:
                    # final chunk: second projection borrows a logits
                    # bank so both output copies run concurrently.
                    big = pl.tile([128, GRP, QC], F32, tag="lg", name="pst")
                    vps = big[:, 0, :]
                else:
                    vps = pp.tile([128, QC], F32, tag="pp",
                                  name=f"psv{qi}{oc}")
                _mm(nc, vps, wvT_sb[:, oc, :], x0a, True, True)
                # residual fused into the PSUM->SBUF copy (DVE); the PE
                # is the bottleneck so no identity-matmul offload.
                ot = outp.tile([128, QC], F32)
                nc.vector.tensor_add(ot, vps, xbf_sb[:, oc, ts(qi, QC)])
                nc.sync.dma_start(out=outr[:, oc, ts(qi, QC)], in_=ot)

            # ---- main loop over query chunks ----
            for qi in range(NQC):
                # row 0: softmax denominator (ones column in hT);
                # rows 1-32: x0 channels.
                x0 = px0.tile([CI + 1, QC], F32)
                x0q = []
                for gi, g0 in enumerate(range(0, NKC, GRP)):
                    ps = pl.tile([128, GRP, QC], F32, tag="lg")
                    eT = eTp.tile([128, GRP, QC], BF16)
                    for j in range(GRP):
                        kc = g0 + j
                        # row-packed: strip kc%4 holds its own copy of
                        # g/f, so adjacent matmuls execute concurrently
                        # in different PE row bands.
                        s = kc % 4
                        sl = slice(32 * s, 32 * (s + 1))
                        nc.tensor.matmul(
                            ps[:, j, :],
                            lhsT=g_sb[sl, ts(kc, KC)],
                            rhs=f_sb[sl, ts(qi, QC)],
                            start=True, stop=True,
                            tile_position=(32 * s, 0),
                        )
                    if gi in DVE_GROUPS:
                        # Schraudolph fast-exp on DVE: bf16 bits of
                        # exp(l) ~= int16(l*EXP_A + EXP_B)
                        nc.vector.tensor_scalar(
                            out=eT.bitcast(I16), in0=ps,
                            scalar1=EXP_A, scalar2=EXP_B,
                            op0=MUL, op1=ADD,
                        )
                    else:
                        nc.scalar.activation(out=eT, in_=ps, func=EXP)
                    # software-pipeline the x0 stage: its wait on this
                    # group's exp then overlaps later groups' logits in
                    # the in-order PE stream.
                    x0q.append((g0, eT))
                    if len(x0q) > PIPE:
                        pg0, peT = x0q.pop(0)
                        for j in range(GRP):
                            kc = pg0 + j
                            _mm(nc, x0, hT_sb[:, kc, :], peT[:, j, :],
                                kc == 0, kc == NKC - 1)
                    if gi % FILL_EVERY == FILL_EVERY - 1:
                        fill(1, rows=128)
                    if qi > 0:
                        if gi == 3:
                            emit_out(qi - 1, 0)
                        elif gi == 6:
                            emit_out(qi - 1, 1)
                            del x0a_by_chunk[qi - 1]
                for pg0, peT in x0q:
                    for j in range(GRP):
                        kc = pg0 + j
                        _mm(nc, x0, hT_sb[:, kc, :], peT[:, j, :],
                            kc == 0, kc == NKC - 1)
                if qi == NQC - 1:
                    # keep the HAM window busy through the tail's
                    # reciprocal chain and final projections.
                    fill(3)
                # softmax divide: row 0 of x0 is the denominator
                rcp = smallp.tile([1, QC], F32, tag="rcp")
                nc.vector.reciprocal_approx_fast(out=rcp, in_=x0[0:1, :])
                rcp_b = smallp.tile([CI + 1, QC], F32, tag="rcpb")
                nc.gpsimd.partition_broadcast(rcp_b, rcp)
                x0a = smallp.tile([CI + 1, QC], BF16, tag="x0a")
                nc.vector.tensor_mul(x0a, x0, rcp_b)
                x0a_by_chunk[qi] = x0a
            emit_out(NQC - 1, 0, tail=True)
            emit_out(NQC - 1, 1, tail=True)

    nc.compile()
    return nc


def kernel(x, wf, bf, wg, bg, wh, bh, wv, bv, gamma):
    global _cached_nc, LAST_EXEC_NS
    if _cached_nc is None:
        _cached_nc = _build()
    nc = _cached_nc

    x = np.asarray(x, dtype=np.float32)
    wf = np.asarray(wf, dtype=np.float32)
    bf = np.asarray(bf, dtype=np.float32)
    wg = np.asarray(wg, dtype=np.float32)
    bg = np.asarray(bg, dtype=np.float32)
    wh = np.asarray(wh, dtype=np.float32)
    bh = np.asarray(bh, dtype=np.float32)
    wv = np.asarray(wv, dtype=np.float32)
    bv = np.asarray(bv, dtype=np.float32)
    g0 = float(np.asarray(gamma, dtype=np.float32).reshape(-1)[0])

    bf16 = ml_dtypes.bfloat16
    xf = np.ascontiguousarray(x.reshape(B, C, N))
    # f/g weights replicated 4x along M so f/g land replicated on the
    # four 32-partition strips (enables row-packed logits matmuls).
    wfT = np.ascontiguousarray(np.tile(wf.T, (1, 4))).astype(bf16)
    wgT = np.ascontiguousarray(np.tile(wg.T, (1, 4))).astype(bf16)
    whT = np.ascontiguousarray(wh.T).astype(bf16)
    wvT = np.empty((CI + 1, C), np.float32)              # aug: bias row 0
    wvT[0, :] = g0 * (bv + wv @ bh)
    wvT[1:, :] = g0 * wv.T
    wvT = wvT.astype(bf16)
    wid = np.eye(128, dtype=np.float32).astype(bf16)
    bf4 = np.ascontiguousarray(np.tile(bf, 4).reshape(128, 1))
    bg4 = np.ascontiguousarray(np.tile(bg, 4).reshape(128, 1))

    in_maps = []
    for core in range(NCORES):
        b, half = divmod(core, 2)
        xb = xf[b]
        if half:
            xb = np.concatenate([xb[:, NQ:], xb[:, :NQ]], axis=1)
        in_maps.append(
            {"xbf": np.ascontiguousarray(xb.astype(bf16)), "wfT": wfT,
             "wgT": wgT, "whT": whT, "wvT": wvT, "wid": wid,
             "bf": bf4, "bg": bg4}
        )

    res = run_bass_kernel_spmd(
        nc, in_maps, list(range(NCORES)),
        trace=TRACE or bool(os.environ.get("BASS_KERNEL_TRACE")),
    )
    LAST_EXEC_NS = res.exec_time_ns

    out = np.empty((B, C, N), np.float32)
    for core in range(NCORES):
        b, half = divmod(core, 2)
        out[b][:, half * NQ : (half + 1) * NQ] = res.results[core]["out"]
    return out.reshape(B, C, W, H)
